# revision 1
# baseline (speedup 1.0000x reference)
"""Trainium2 Bass kernel for DecoderSplattingCUDA (EWA Gaussian splatting).

Contract: kernel(**inputs) takes the FULL inputs of reference.setup_inputs()
and returns the FULL [b, v, 3, H, W] image, computed on 8 NeuronCores.

Layout: gaussians on partitions (depth sorted), pixels on the free axis.
The image is split into 32 (camera, 8-row band) pairs, striped across the 8
cores (4 bands per core) for load balance.  Per band the host culls the
gaussians that can reach alpha >= 1/255 inside the band's y-range (the
reference's own cull threshold, applied conservatively, so results stay
exact) and pads the survivor list to BPAD blocks of 128.

Per (band, block) on a [128 g, 1024 px] tile:
  s = X + e_row          (vector tensor_scalar per row, e = r*dy - u)
  q1 = (gamma*s)^2       (scalar Square, per-partition scale)
  D = max(q1 - bias, -ln(.99)), bias = logop - (delta*dy)^2  (vector dual-op)
  alpha0 = exp(-D)       (scalar)
  m = D <= ln(255)       (gpsimd; the alpha < 1/255 cull)
  alpha = alpha0 * m     (vector)
  lga = ln(1 - alpha)    (scalar, fp16 out)
Depth-ordered transmittance T_g = exp(cumsum lga) is a triangular-ones fp16
matmul per block; carries across blocks come from a staircase matmul
accumulated over the band's blocks and broadcast back with selector-matrix
matmuls (error-compensated fp16 hi+lo pair).  The composite uses summation
by parts: img = c_0 + sum_g (c_{g+1}-c_g) T_g with c_G := background, so the
color matmul contracts T directly and the background term is free.
"""
import os
import sys

sys.path.insert(0, "/opt/trn_rl_repo/concourse")

from contextlib import ExitStack

import numpy as np

import concourse.bacc as bacc
import concourse.tile as tile
from concourse import mybir
from concourse.bass_utils import run_bass_kernel_spmd
from concourse.hw_specs import get_activation_tables

F32 = mybir.dt.float32
F16 = mybir.dt.float16
AF = mybir.ActivationFunctionType
ALU = mybir.AluOpType

C0 = 0.28209479177387814
C1 = 0.4886025119029199
NEAR, FAR = 0.1, 1000.0

H = W = 128
G = 2048               # gaussians per camera (2 * 32 * 32)
NCAM = 2
BAND_ROWS = 8          # image rows per band
NBAND = H // BAND_ROWS          # bands per camera (16)
NPAIR = NCAM * NBAND            # (camera, band) pairs (32)
NSLOT = NPAIR // 8              # pairs per core (4)
BPX = BAND_ROWS * W             # pixels per band (1024)
PT = 512                        # matmul free-dim tile
NPT = BPX // PT                 # pixel tiles per band (2)

LN99 = float(np.float32(-np.log(np.float32(0.99))))     # 0.01005034
LN255 = float(np.float32(np.log(np.float32(255.0))))    # 5.5412636
NEG_BIG = -200.0
SQ_ENGINE = os.environ.get("SPLAT_SQ", "act")  # act|gpsimd|vector

_NC_CACHE = {}
_LAST_EXEC_NS = None
_LAST_RESULTS = None


def _only_full_act_set(arch):
    """Steer insert_act_table_loads to the one table set that covers
    Square+Exp+Ln+Copy+Identity (natural_log_exp_and_others), so the kernel
    pays a single ACT table load instead of one per function switch.  Other
    sets are emptied but keep their list position, so act_func_set_id
    indices still match walrus's act_info.json."""
    full = get_activation_tables(arch)
    keep = "natural_log_exp_and_others"
    return {name: (fns if name == keep else set()) for name, fns in full.items()}


# ---------------------------------------------------------------- host prep
def _prep_camera(extr, K, bg, means, cov, sh, op):
    """Mirror of reference._render_one's per-gaussian math (numpy f32).
    Returns depth-sorted per-gaussian arrays."""
    f32 = np.float32
    extr = extr.astype(f32)
    try:
        w2c = np.linalg.inv(extr.astype(np.float64)).astype(f32)
    except np.linalg.LinAlgError:
        w2c = np.linalg.pinv(extr.astype(np.float64)).astype(f32)
    R, t = w2c[:3, :3], w2c[:3, 3]
    p = means @ R.T + t
    x, y, z = p[:, 0], p[:, 1], p[:, 2]
    zc = np.maximum(z, f32(1e-6))
    fx, fy = K[0, 0], K[1, 1]
    cx, cy = K[0, 2], K[1, 2]
    u = fx * x / zc + cx
    v = fy * y / zc + cy
    cov_c = np.einsum("ij,gjk,lk->gil", R, cov, R)
    zero = np.zeros_like(zc)
    J = np.stack([np.stack([fx / zc, zero, -fx * x / (zc * zc)], -1),
                  np.stack([zero, fy / zc, -fy * y / (zc * zc)], -1)], -2)
    cov2d = np.einsum("gij,gjk,glk->gil", J, cov_c, J)
    a = cov2d[:, 0, 0] + f32(0.3)
    bb = cov2d[:, 0, 1]
    c = cov2d[:, 1, 1] + f32(0.3)
    det = np.maximum(a * c - bb * bb, f32(1e-12))
    ia, ib, ic = c / det, -bb / det, a / det
    # SH degree-1 -> RGB
    d = means - extr[:3, 3]
    d = d / np.linalg.norm(d, axis=-1, keepdims=True)
    col = C0 * sh[:, :, 0]
    if sh.shape[-1] >= 4:
        col = (col - C1 * d[:, 1:2] * sh[:, :, 1]
               + C1 * d[:, 2:3] * sh[:, :, 2]
               - C1 * d[:, 0:1] * sh[:, :, 3])
    col = np.maximum(col + f32(0.5), f32(0.0)).astype(f32)  # [G, 3]

    valid = (z > f32(NEAR)) & (z < f32(FAR))
    op_eff = np.where(valid, op, f32(0.0))

    order = np.argsort(z, kind="stable")
    u, v, ia, ib, ic, op_eff, z = (arr[order] for arr in
                                   (u, v, ia, ib, ic, op_eff, z))
    col = col[order]

    # completed square: power = -sa*(gamma*(dx + r*dy))^2 - se*(delta*dy)^2
    psd = bool(np.all(ia > 0))
    with np.errstate(divide="ignore", invalid="ignore"):
        r = np.where(ia != 0, ib / ia, f32(0.0)).astype(f32)
        eta = ic - np.where(ia != 0, ib * ib / ia, f32(0.0))
        gamma = np.sqrt(np.abs(ia) * f32(0.5)).astype(f32)
        delta = np.sqrt(np.abs(eta) * f32(0.5)).astype(f32)
        logop = np.where(op_eff > 0, np.log(np.maximum(op_eff, f32(1e-30))),
                         f32(NEG_BIG))
    logop = np.maximum(logop, f32(NEG_BIG)).astype(f32)
    sa = np.sign(ia).astype(f32)
    sa[sa == 0] = 1.0
    se = np.sign(eta).astype(f32)
    se[se == 0] = 1.0
    psd = psd and bool(np.all(eta > 0))
    return dict(u=u.astype(f32), v=v.astype(f32), r=r, gamma=gamma,
                delta=delta, logop=logop, sa=sa, se=se, col=col,
                psd=psd, psd_g=(ia > 0) & (eta > 0))


def _cull_band(cp, band, bg):
    """Indices (in sorted order) of gaussians that can reach alpha >= 1/255
    anywhere in the band; conservative, so dropped ones are exactly zero in
    the reference too.  Returns (idx, dc[3 per kept], c0[3])."""
    f32 = np.float32
    ylo = f32(band * BAND_ROWS + 0.5)
    yhi = f32(band * BAND_ROWS + BAND_ROWS - 0.5)
    v = cp["v"]
    dymin = np.maximum(0.0, np.maximum(ylo - v, v - yhi)).astype(f32)
    reach = (cp["delta"] * dymin) ** 2 <= cp["logop"] + f32(LN255 + 0.01)
    keep = reach | ~cp["psd_g"]     # non-PSD conics: never cull
    idx = np.nonzero(keep)[0]
    col = cp["col"][idx]
    n = len(idx)
    dc = np.zeros((n, 3), f32)
    if n:
        dc[:-1] = col[1:] - col[:-1]
        dc[-1] = bg - col[-1]
        c0 = col[0].copy()
    else:
        c0 = bg.astype(f32).copy()
    return idx, dc, c0


# ------------------------------------------------------------- bass program
def _build_nc(general: bool, bpads: tuple):
    nc = bacc.Bacc(None, target_bir_lowering=False)

    NSC = 8  # per-block scalars: u, r, gamma, v, delta, logop, sa, -se
    NBLK = sum(bpads)
    mb = max(bpads)
    koff = [sum(bpads[:i]) for i in range(NSLOT)]
    gs_d = nc.dram_tensor("gs", [128, NBLK * NSC], F32, kind="ExternalInput")
    dc_d = nc.dram_tensor("dcw", [128, NBLK * 6], F16, kind="ExternalInput")
    x128_d = nc.dram_tensor("x128", [128, 128], F32, kind="ExternalInput")
    yc_d = nc.dram_tensor("yc", [128, NSLOT * BAND_ROWS], F32,
                          kind="ExternalInput")
    u128_d = nc.dram_tensor("u128", [128, 128], F16, kind="ExternalInput")
    eb_d = nc.dram_tensor("eb", [128, mb * 128], F16, kind="ExternalInput")
    st_d = nc.dram_tensor("st", [128, mb * mb], F16, kind="ExternalInput")
    img_d = nc.dram_tensor("img", [3, NSLOT * BPX], F32, kind="ExternalOutput")

    with tile.TileContext(nc) as tc, ExitStack() as ctx:
        consts = ctx.enter_context(tc.tile_pool(name="consts", bufs=1))
        prep = ctx.enter_context(tc.tile_pool(name="prep", bufs=1))
        work = ctx.enter_context(tc.tile_pool(name="work", bufs=3))
        lgap = ctx.enter_context(tc.tile_pool(name="lgap", bufs=2 * mb))
        carp = ctx.enter_context(tc.tile_pool(name="carp", bufs=2))
        outp = ctx.enter_context(tc.tile_pool(name="outp", bufs=2))
        psum = ctx.enter_context(tc.tile_pool(name="psum", bufs=1, space="PSUM"))
        psum2 = ctx.enter_context(tc.tile_pool(name="psum2", bufs=1, space="PSUM"))
        scanp = ctx.enter_context(tc.tile_pool(name="scanp", bufs=1, space="PSUM"))

        gs = consts.tile([128, NBLK * NSC], F32)
        dcw = consts.tile([128, NBLK * 6], F16)
        x128 = consts.tile([128, 128], F32)
        yc = consts.tile([128, NSLOT * BAND_ROWS], F32)
        u128 = consts.tile([128, 128], F16)
        eb = consts.tile([128, mb * 128], F16)
        st = consts.tile([128, mb * mb], F16)
        for t, d in ((gs, gs_d), (dcw, dc_d), (x128, x128_d), (yc, yc_d),
                     (u128, u128_d), (eb, eb_d), (st, st_d)):
            nc.gpsimd.dma_start(t[:], d[:])

        def S(k, j):  # per-partition scalar AP for flat block k, slot j
            return gs[:, k * NSC + j: k * NSC + j + 1]

        # per-(block,row) prep: e = r*dy - u ; bias = logop - se*(delta*dy)^2
        eM, biasM = [], []
        for k in range(NBLK):
            sl = max(i for i in range(NSLOT) if koff[i] <= k)
            ys = yc[:, sl * BAND_ROWS:(sl + 1) * BAND_ROWS]
            dyM = prep.tile([128, BAND_ROWS], F32, tag="dyM")
            nc.vector.tensor_scalar(dyM[:], ys, S(k, 3), None, ALU.subtract)
            e = prep.tile([128, BAND_ROWS], F32, tag=f"eM{k}")
            nc.vector.tensor_scalar(e[:], dyM[:], S(k, 1), S(k, 0),
                                    ALU.mult, ALU.subtract)
            tq = prep.tile([128, BAND_ROWS], F32, tag="tqM")
            nc.scalar.activation(tq[:], dyM[:], AF.Square, scale=S(k, 4))
            bias = prep.tile([128, BAND_ROWS], F32, tag=f"biasM{k}")
            if general:
                nc.vector.tensor_scalar(bias[:], tq[:], S(k, 7), S(k, 5),
                                        ALU.mult, ALU.add)
            else:
                nc.vector.tensor_scalar(bias[:], tq[:], S(k, 5), -1.0,
                                        ALU.subtract, ALU.mult)
            eM.append(e)
            biasM.append(bias)

        BASE = [0, 64]          # ptile partition bases within shared banks

        def emit_A(sl):
            """Phase A for slot sl, one block per yield."""
            bpad = bpads[sl]
            ps_c = psum2.tile([128, PT], F32, tag=f"ps_c{sl % 2}",
                              name=f"ps_c{sl}")
            lgas = []
            for b in range(bpad):
                k = koff[sl] + b
                s = work.tile([128, BPX], F32, tag="s", bufs=2)
                for rr in range(BAND_ROWS):
                    nc.vector.tensor_scalar(
                        s[:, rr * 128:(rr + 1) * 128], x128[:],
                        eM[k][:, rr:rr + 1], None, ALU.add)
                q1 = work.tile([128, BPX], F32, tag="q1", bufs=2)
                nc.scalar.activation(q1[:], s[:], AF.Square, scale=S(k, 2))
                D = work.tile([128, BPX], F32, tag="D")
                if general:
                    Draw = work.tile([128, BPX], F32, tag="Draw")
                    for rr in range(BAND_ROWS):
                        nc.vector.tensor_scalar(
                            Draw[:, rr * 128:(rr + 1) * 128],
                            q1[:, rr * 128:(rr + 1) * 128],
                            S(k, 6), biasM[k][:, rr:rr + 1],
                            ALU.mult, ALU.subtract)
                    nc.vector.tensor_scalar(D[:], Draw[:], LN99, None, ALU.max)
                else:
                    for rr in range(BAND_ROWS):
                        nc.vector.tensor_scalar(
                            D[:, rr * 128:(rr + 1) * 128],
                            q1[:, rr * 128:(rr + 1) * 128],
                            biasM[k][:, rr:rr + 1], LN99,
                            ALU.subtract, ALU.max)
                alpha0 = work.tile([128, BPX], F32, tag="alpha0")
                nc.scalar.activation(alpha0[:], D[:], AF.Exp, scale=-1.0)
                m = work.tile([128, BPX], F32, tag="m")
                nc.gpsimd.tensor_scalar(m[:], D[:], LN255, None, ALU.is_le)
                alpha = work.tile([128, BPX], F32, tag="alpha")
                nc.vector.tensor_tensor(alpha[:], alpha0[:], m[:], ALU.mult)
                if general:
                    m2 = work.tile([128, BPX], F32, tag="m2")
                    nc.vector.tensor_scalar(m2[:], Draw[:], S(k, 5), 0.0,
                                            ALU.add, ALU.is_ge)
                    alpha2 = work.tile([128, BPX], F32, tag="alpha2")
                    nc.vector.tensor_tensor(alpha2[:], alpha[:], m2[:],
                                            ALU.mult)
                    alpha = alpha2
                lga = lgap.tile([128, BPX], F16, tag="lga")
                nc.scalar.activation(lga[:], alpha[:], AF.Ln,
                                     scale=-1.0, bias=1.0)
                lgas.append(lga)
                for pt in range(NPT):
                    base = BASE[pt]
                    nc.tensor.matmul(
                        ps_c[base:base + bpad, :],
                        st[:, mb * b:mb * b + bpad],
                        lga[:, PT * pt:PT * (pt + 1)],
                        start=(b == 0), stop=(b == bpad - 1),
                        tile_position=(0, base))
                yield
            # phase B: compensated fp16 carries (hi+lo)
            ch = carp.tile([128, PT], F16, tag="c16h")
            nc.vector.tensor_copy(ch[:], ps_c[:])
            chf = carp.tile([128, PT], F32, tag="c16hf")
            nc.vector.tensor_copy(chf[:], ch[:])
            rs = carp.tile([128, PT], F32, tag="res")
            nc.vector.tensor_tensor(rs[:], ps_c[:], chf[:], ALU.subtract)
            cl = carp.tile([128, PT], F16, tag="c16l")
            nc.vector.tensor_copy(cl[:], rs[:])
            state[sl] = (lgas, ch, cl)

        def emit_C(sl):
            """Phase C + D for slot sl, one block per yield."""
            bpad = bpads[sl]
            lgas, ch, cl = state[sl]
            img_ps = psum.tile([128, PT], F32, tag=f"img{sl % 2}",
                               name=f"img{sl}")
            for b in range(bpad):
                k = koff[sl] + b
                ps_s = scanp.tile([128, BPX], F32, tag=f"scan{b % 2}",
                                  name=f"scan{sl}_{b}")
                for pt in range(NPT):
                    base = BASE[pt]
                    sel = eb[base:base + bpad, 128 * b:128 * (b + 1)]
                    nc.tensor.matmul(ps_s[:, pt * PT:(pt + 1) * PT],
                                     u128[:],
                                     lgas[b][:, PT * pt:PT * (pt + 1)],
                                     start=True, stop=False)
                    nc.tensor.matmul(ps_s[:, pt * PT:(pt + 1) * PT], sel,
                                     ch[base:base + bpad, :],
                                     start=False, stop=False)
                    nc.tensor.matmul(ps_s[:, pt * PT:(pt + 1) * PT], sel,
                                     cl[base:base + bpad, :],
                                     start=False, stop=True)
                exT = work.tile([128, BPX], F16, tag="exT")
                nc.scalar.activation(exT[:], ps_s[:], AF.Exp)
                for pt in range(NPT):
                    base = BASE[pt]
                    nc.tensor.matmul(
                        img_ps[base:base + 3, :],
                        dcw[:, 6 * k:6 * k + 3],
                        exT[:, pt * PT:(pt + 1) * PT],
                        start=(b == 0), stop=False,
                        tile_position=(0, base))
                    nc.tensor.matmul(
                        img_ps[base:base + 3, :],
                        dcw[:, 6 * k + 3:6 * k + 6],
                        exT[:, pt * PT:(pt + 1) * PT],
                        start=False, stop=(b == bpad - 1),
                        tile_position=(0, base))
                yield
            for pt in range(NPT):
                base = BASE[pt]
                ob = outp.tile([128, PT], F32, tag="ob")
                nc.vector.tensor_copy(ob[base:base + 3, :],
                                      img_ps[base:base + 3, :])
                nc.sync.dma_start(
                    img_d[:, (sl * NPT + pt) * PT:(sl * NPT + pt + 1) * PT],
                    ob[base:base + 3, :])

        # software-pipelined emission: C(sl-1) interleaves with A(sl) so the
        # scheduler (priority ~ emission order) overlaps PE/ACT phase C work
        # with DVE/ACT phase A work of the next slot.
        state = {}
        prev_c = None
        for sl in range(NSLOT):
            for _ in emit_A(sl):
                if prev_c is not None:
                    next(prev_c, None)
            if prev_c is not None:
                for _ in prev_c:    # drain remaining C blocks + phase D
                    pass
            prev_c = emit_C(sl)
        for _ in prev_c:
            pass

    saved = bacc.get_activation_tables
    bacc.get_activation_tables = _only_full_act_set
    try:
        nc.compile()
    finally:
        bacc.get_activation_tables = saved
    return nc


# ------------------------------------------------------------------ driver
def kernel(context_pose, target_poses, target_intrinsics, means1, means2,
           cov1, cov2, sh1, sh2, op1, op2, background_color,
           image_h, image_w):
    f32 = np.float32
    b, v = np.asarray(target_poses).shape[:2]
    assert b == 1 and v == NCAM and int(image_h) == H and int(image_w) == W

    context_pose = np.asarray(context_pose, f32)
    target_poses = np.asarray(target_poses, f32)
    target_intrinsics = np.asarray(target_intrinsics, f32)
    bg = np.asarray(background_color, f32)

    try:
        inv_base = np.linalg.inv(
            context_pose[0].astype(np.float64)).astype(f32)
    except np.linalg.LinAlgError:
        inv_base = np.linalg.pinv(
            context_pose[0].astype(np.float64)).astype(f32)
    d_sh = np.asarray(sh1).shape[-1]
    means = np.stack([np.asarray(means1, f32), np.asarray(means2, f32)],
                     1).reshape(-1, 3)
    covs = np.stack([np.asarray(cov1, f32), np.asarray(cov2, f32)],
                    1).reshape(-1, 3, 3)
    shs = np.stack([np.asarray(sh1, f32), np.asarray(sh2, f32)],
                   1).reshape(-1, 3, d_sh)
    ops = np.stack([np.asarray(op1, f32), np.asarray(op2, f32)],
                   1).reshape(-1)
    assert means.shape[0] == G

    row_scale = np.array([1.0 / W, 1.0 / H, 1.0], f32)[:, None]

    cams = []
    for cam in range(NCAM):
        extr = inv_base @ target_poses[0, cam]
        Kn = target_intrinsics[0, cam] * row_scale
        K = np.array([[Kn[0, 0] * W, 0, Kn[0, 2] * W],
                      [0, Kn[1, 1] * H, Kn[1, 2] * H],
                      [0, 0, 1]], f32)
        cams.append(_prep_camera(extr, K, bg, means, covs, shs, ops))
    general = not all(c["psd"] for c in cams)

    # cull per (camera, band) pair, then group the 32 pairs by survivor
    # count into NSLOT groups of 8 (one per core): slot j runs the j-th
    # largest group, so padding is per-group, not global max.
    pairs = []
    for p in range(NPAIR):
        cam, band = divmod(p, NBAND)
        idx, dc, c0 = _cull_band(cams[cam], band, bg)
        pairs.append((cam, band, idx, dc, c0))
    order = sorted(range(NPAIR), key=lambda p: -len(pairs[p][2]))
    assign = [[order[g * 8 + i] for i in range(8)] for g in range(NSLOT)]
    bpads = tuple(max(1, -(-max(len(pairs[p][2]) for p in grp) // 128))
                  for grp in assign)

    key = (bool(general), bpads)
    if key not in _NC_CACHE:
        _NC_CACHE[key] = _build_nc(general, bpads)
    nc = _NC_CACHE[key]
    mb = max(bpads)
    koff = [sum(bpads[:i]) for i in range(NSLOT)]

    # shared constants
    x128 = np.broadcast_to(np.arange(W, dtype=f32) + 0.5, (128, W)).copy()
    u128 = np.triu(np.ones((128, 128), np.float16))          # k <= j
    st = np.zeros((128, mb * mb), np.float16)                # j > b staircase
    for b_ in range(mb):
        st[:, mb * b_ + b_ + 1:mb * (b_ + 1)] = 1.0
    ebm = np.zeros((128, mb * 128), np.float16)              # carry selector
    for b_ in range(mb):
        ebm[b_, b_ * 128:(b_ + 1) * 128] = 1.0
        ebm[64 + b_, b_ * 128:(b_ + 1) * 128] = 1.0

    NSC = 8
    NBLK = sum(bpads)
    in_maps = []
    for core in range(8):
        gs = np.zeros((128, NBLK * NSC), f32)
        dc16 = np.zeros((128, NBLK * 6), np.float16)
        ycv = np.zeros(NSLOT * BAND_ROWS, f32)
        for slot in range(NSLOT):
            bpad = bpads[slot]
            cam, band, idx, dc, c0 = pairs[assign[slot][core]]
            cp = cams[cam]
            n = len(idx)
            ycv[slot * BAND_ROWS:(slot + 1) * BAND_ROWS] = (
                np.arange(BAND_ROWS, dtype=f32) + band * BAND_ROWS + 0.5)
            arrs = {j: cp[nm][idx] for j, nm in enumerate(
                ("u", "r", "gamma", "v", "delta", "logop", "sa"))}
            nse = -cp["se"][idx]
            dch = dc.astype(np.float16)
            dcl = (dc - dch.astype(f32)).astype(np.float16)
            for b_ in range(bpad):
                kf = koff[slot] + b_
                lo, hi = b_ * 128, min(n, (b_ + 1) * 128)
                cnt = max(0, hi - lo)
                if cnt > 0:
                    for j in range(7):
                        gs[:cnt, kf * NSC + j] = arrs[j][lo:hi]
                    gs[:cnt, kf * NSC + 7] = nse[lo:hi]
                    dc16[:cnt, kf * 6:kf * 6 + 3] = dch[lo:hi]
                    dc16[:cnt, kf * 6 + 3:kf * 6 + 6] = dcl[lo:hi]
                # padding rows: logop = NEG_BIG (alpha = 0), gamma/delta 1
                if cnt < 128:
                    gs[cnt:, kf * NSC + 2] = 1.0
                    gs[cnt:, kf * NSC + 4] = 1.0
                    gs[cnt:, kf * NSC + 5] = NEG_BIG
                    gs[cnt:, kf * NSC + 6] = 1.0
                    gs[cnt:, kf * NSC + 7] = -1.0
        yc = np.broadcast_to(ycv, (128, NSLOT * BAND_ROWS)).copy()
        in_maps.append({"gs": gs, "dcw": dc16, "x128": x128, "yc": yc,
                        "u128": u128, "eb": ebm, "st": st})

    trace = os.environ.get("SPLAT_TRACE", "0") == "1"
    res = run_bass_kernel_spmd(nc, in_maps, core_ids=list(range(8)),
                               trace=trace,
                               trace_cores=list(range(8)) if trace else None)
    global _LAST_EXEC_NS, _LAST_RESULTS
    _LAST_EXEC_NS = res.exec_time_ns
    _LAST_RESULTS = res

    out = np.zeros((1, NCAM, 3, H, W), f32)
    for core in range(8):
        img = res.results[core]["img"]
        for slot in range(NSLOT):
            cam, band, idx, dc, c0 = pairs[assign[slot][core]]
            piece = img[:, slot * BPX:(slot + 1) * BPX].reshape(
                3, BAND_ROWS, W)
            out[0, cam, :, band * BAND_ROWS:(band + 1) * BAND_ROWS, :] = (
                piece + c0[:, None, None])
    return out



# revision 3
# speedup vs baseline: 4.0290x; 4.0290x over previous
"""Trainium2 Bass kernel for DecoderSplattingCUDA (EWA Gaussian splatting).

Contract: kernel(**inputs) takes the FULL inputs of reference.setup_inputs()
and returns the FULL [b, v, 3, H, W] image, computed on 8 NeuronCores.

Layout (v4): PIXELS on partitions, gaussians along the free axis.
The image is cut into 256 tiles of 8x16 = 128 pixels (one partition per
pixel).  Per tile the host culls gaussians by their exact peak alpha and
emits, per survivor, the 6 coefficients of the screen-space quadratic
  D(x,y) = A x~^2 + B x~y~ + C y~^2 + Dx x~ + Ey y~ + F   (tile-centered)
with alpha = exp(-D) already folding in opacity (F includes -log(op)).

Device per (tile batch = slot of <=1024 survivor columns):
  D     = matmul(mono[6,128]^T, coeff[6,L])   PE, fp16 hi+lo (exact-ish)
  alpha = Exp(-D)                              ACT, psum -> sbuf fp16
  mcull = alpha < 1/255                        Pool
  na    = 1 - alpha  (max 0.01 if clamp slot)  DVE dual-op
  om    = max(na, mcull)                       DVE   (culled -> om = 1)
  T     = tensor_tensor_scan(om, mult)         DVE, per tile, init 1.0
  Tt    = PE transpose per 128-col chunk -> psum fp16 -> sbuf
  img^T[128px,3] += Tt_chunk^T @ dc[128,3]     PE, accumulated per tile
Host adds the summation-by-parts constant c1 per tile and reassembles.
T_g = prod_{i<=g}(1-alpha_i) exactly matches the reference compositing
order (depth-sorted survivor lists), with img = c1 + sum_g T_g dc_g.
"""
import os
import sys

sys.path.insert(0, "/opt/trn_rl_repo/concourse")

from contextlib import ExitStack

import numpy as np

import concourse.bacc as bacc
import concourse.tile as tile
from concourse import mybir
from concourse.bass_utils import run_bass_kernel_spmd
from concourse.hw_specs import get_activation_tables

F32 = mybir.dt.float32
F16 = mybir.dt.float16
AF = mybir.ActivationFunctionType
ALU = mybir.AluOpType

C0 = 0.28209479177387814
C1 = 0.4886025119029199
NEAR, FAR = 0.1, 1000.0
LN255 = float(np.float32(np.log(np.float32(255.0))))
NEG_BIG = -200.0

H = W = 128
NCAM = 2
TR, TC = 8, 16                  # tile shape (rows x cols) = 128 px
NTY, NTX = H // TR, W // TC     # 16 x 8 tiles per camera
NTILE = NCAM * NTY * NTX        # 256
NPC = NTILE // 8                # tiles per core (32)
SLOT_CAP = 1024                 # max survivor columns per slot (psum banks)
PAD_F = 30000.0                 # padding column: D = PAD_F -> alpha = 0

_NC_CACHE = {}
_LAST_EXEC_NS = None
_LAST_RESULTS = None


def _only_full_act_set(arch):
    full = get_activation_tables(arch)
    keep = "natural_log_exp_and_others"
    return {name: (fns if name == keep else set()) for name, fns in full.items()}


# ---------------------------------------------------------------- host prep
def _prep_camera(extr, K, means, cov, sh, op):
    """Per-gaussian camera math (numpy f32), depth-sorted."""
    f32 = np.float32
    extr = extr.astype(f32)
    try:
        w2c = np.linalg.inv(extr.astype(np.float64)).astype(f32)
    except np.linalg.LinAlgError:
        w2c = np.linalg.pinv(extr.astype(np.float64)).astype(f32)
    R, t = w2c[:3, :3], w2c[:3, 3]
    p = means @ R.T + t
    x, y, z = p[:, 0], p[:, 1], p[:, 2]
    zc = np.maximum(z, f32(1e-6))
    fx, fy = K[0, 0], K[1, 1]
    cx, cy = K[0, 2], K[1, 2]
    u = fx * x / zc + cx
    v = fy * y / zc + cy
    cov_c = np.einsum("ij,gjk,lk->gil", R, cov, R)
    zero = np.zeros_like(zc)
    J = np.stack([np.stack([fx / zc, zero, -fx * x / (zc * zc)], -1),
                  np.stack([zero, fy / zc, -fy * y / (zc * zc)], -1)], -2)
    cov2d = np.einsum("gij,gjk,glk->gil", J, cov_c, J)
    a = cov2d[:, 0, 0] + f32(0.3)
    bb = cov2d[:, 0, 1]
    c = cov2d[:, 1, 1] + f32(0.3)
    det = np.maximum(a * c - bb * bb, f32(1e-12))
    ia, ib, ic = c / det, -bb / det, a / det
    d = means - extr[:3, 3]
    d = d / np.linalg.norm(d, axis=-1, keepdims=True)
    col = C0 * sh[:, :, 0]
    if sh.shape[-1] >= 4:
        col = (col - C1 * d[:, 1:2] * sh[:, :, 1]
               + C1 * d[:, 2:3] * sh[:, :, 2]
               - C1 * d[:, 0:1] * sh[:, :, 3])
    col = np.maximum(col + f32(0.5), f32(0.0)).astype(f32)

    valid = (z > f32(NEAR)) & (z < f32(FAR))
    op_eff = np.where(valid, op, f32(0.0))
    order = np.argsort(z, kind="stable")
    u, v, ia, ib, ic, op_eff = (arr[order] for arr in
                                (u, v, ia, ib, ic, op_eff))
    col = col[order]

    psd_g = (ia > 0) & (ic - np.where(ia != 0, ib * ib / ia, 0.0) > 0)
    with np.errstate(divide="ignore", invalid="ignore"):
        r = np.where(ia != 0, ib / ia, f32(0.0)).astype(f32)
        eta = ic - np.where(ia != 0, ib * ib / ia, f32(0.0))
        gamma2 = (np.abs(ia) * f32(0.5)).astype(f32)
        delta2 = (np.abs(eta) * f32(0.5)).astype(f32)
        logop = np.where(op_eff > 0, np.log(np.maximum(op_eff, f32(1e-30))),
                         f32(NEG_BIG))
    logop = np.maximum(logop, f32(NEG_BIG)).astype(f32)
    return dict(u=u.astype(f32), v=v.astype(f32), r=r, gamma2=gamma2,
                delta2=delta2, logop=logop, col=col,
                psd=bool(np.all(psd_g)))


def _tile_data(cp, ty, tx, bg):
    """Exact cull for tile (ty, tx); returns per-survivor coeffs, dc, c1,
    and the max unclamped alpha (for the 0.99-clamp flag)."""
    f32 = np.float32
    r0, c0 = ty * TR, tx * TC
    u, v, r = cp["u"], cp["v"], cp["r"]
    g2, d2, logop = cp["gamma2"], cp["delta2"], cp["logop"]
    # conservative candidate box test
    ylo, yhi = f32(r0 + 0.5), f32(r0 + TR - 0.5)
    xlo, xhi = f32(c0 + 0.5), f32(c0 + TC - 0.5)
    dymin = np.maximum(0.0, np.maximum(ylo - v, v - yhi)).astype(f32)
    dy_a, dy_b = ylo - v, yhi - v
    x0_a, x0_b = u - r * dy_a, u - r * dy_b
    x0_lo = np.minimum(x0_a, x0_b)
    x0_hi = np.maximum(x0_a, x0_b)
    dxmin = np.maximum(0.0, np.maximum(x0_lo - xhi, xlo - x0_hi)).astype(f32)
    q = d2 * dymin ** 2 + g2 * dxmin ** 2
    cand = np.nonzero(q <= logop + f32(LN255 + 0.02))[0]
    if len(cand) == 0:
        return (np.zeros((6, 0), f32), np.zeros((0, 3), f32),
                bg.astype(f32).copy(), 0.0)
    # exact alpha over the 128 pixels for candidates
    xs = np.arange(c0, c0 + TC, dtype=f32) + 0.5
    ys = np.arange(r0, r0 + TR, dtype=f32) + 0.5
    yy, xx = np.meshgrid(ys, xs, indexing="ij")
    xx, yy = xx.reshape(-1), yy.reshape(-1)
    gu, gv, gr = u[cand, None], v[cand, None], r[cand, None]
    gg2, gd2, glo = g2[cand, None], d2[cand, None], logop[cand, None]
    dx = xx[None, :] - gu
    dyv = yy[None, :] - gv
    D = gg2 * (dx + gr * dyv) ** 2 + gd2 * dyv ** 2 - glo
    amax = np.exp(-np.maximum(D.min(axis=1), 0.0))
    keep = amax >= f32(1.0 / 255.0) - f32(1e-6)
    idx = cand[keep]
    if len(idx) == 0:
        return (np.zeros((6, 0), f32), np.zeros((0, 3), f32),
                bg.astype(f32).copy(), 0.0)
    # tile-centered quadratic coefficients
    x0f, y0f = f32(c0 + TC / 2.0), f32(r0 + TR / 2.0)
    ut, vt = u[idx] - x0f, v[idx] - y0f
    rr, gg, dd, lo = r[idx], g2[idx], d2[idx], logop[idx]
    st = ut + rr * vt
    coef = np.stack([gg,
                     2 * gg * rr,
                     gg * rr * rr + dd,
                     -2 * gg * st,
                     -2 * gg * rr * st - 2 * dd * vt,
                     gg * st * st + dd * vt * vt - lo], 0).astype(f32)
    col = cp["col"][idx]
    n = len(idx)
    dc = np.zeros((n, 3), f32)
    dc[:-1] = col[1:] - col[:-1]
    dc[-1] = bg - col[-1]
    return coef, dc, col[0].copy(), float(amax[keep].max())


# ------------------------------------------------------------- bass program
def _build_nc(struct):
    """struct: dict with
      slots: list of slots; each slot = list of padded tile lengths
      flags: per-slot bool (apply 0.99 clamp)
      novl:  total number of (chunk, tile) overlap color matmuls
      overlaps: per slot: list of (chunk_local_idx, col_lo, col_hi,
                 tile_idx_in_slot, ov_idx, is_first, is_last)
    """
    slots = struct["slots"]
    flags = struct["flags"]
    novl = struct["novl"]
    SL = sum(sum(s) for s in slots)
    nc = bacc.Bacc(None, target_bir_lowering=False)

    mono_d = nc.dram_tensor("mono", [6, 128], F16, kind="ExternalInput")
    ident_d = nc.dram_tensor("ident", [128, 128], F16, kind="ExternalInput")
    chi_d = nc.dram_tensor("chi", [6, SL], F16, kind="ExternalInput")
    clo_d = nc.dram_tensor("clo", [6, SL], F16, kind="ExternalInput")
    dcw_d = nc.dram_tensor("dcw", [128, 3 * novl], F16, kind="ExternalInput")
    img_d = nc.dram_tensor("img", [128, 3 * NPC], F32, kind="ExternalOutput")

    with tile.TileContext(nc) as tc, ExitStack() as ctx:
        consts = ctx.enter_context(tc.tile_pool(name="consts", bufs=1))
        apool = ctx.enter_context(tc.tile_pool(name="apool", bufs=2))
        tpool = ctx.enter_context(tc.tile_pool(name="tpool", bufs=2))
        ttspool = ctx.enter_context(tc.tile_pool(name="ttspool", bufs=3))
        outp = ctx.enter_context(tc.tile_pool(name="outp", bufs=2))
        dmmp = ctx.enter_context(tc.tile_pool(name="dmmp", bufs=2,
                                              space="PSUM"))
        tpp = ctx.enter_context(tc.tile_pool(name="tpp", bufs=3,
                                             space="PSUM"))
        imgp = ctx.enter_context(tc.tile_pool(name="imgp", bufs=1,
                                              space="PSUM"))

        mono = consts.tile([6, 128], F16)
        ident = consts.tile([128, 128], F16)
        chi = consts.tile([6, SL], F16)
        clo = consts.tile([6, SL], F16)
        dcw = consts.tile([128, 3 * novl], F16)
        soffs = []
        off = 0
        for s in slots:
            soffs.append(off)
            off += sum(s)
        nc.gpsimd.dma_start(mono[:], mono_d[:])
        nc.gpsimd.dma_start(ident[:], ident_d[:])
        nc.gpsimd.dma_start(dcw[:], dcw_d[:])
        for si, s in enumerate(slots):
            Ls = sum(s)
            nc.gpsimd.dma_start(chi[:, soffs[si]:soffs[si] + Ls],
                                chi_d[:, soffs[si]:soffs[si] + Ls])
            nc.gpsimd.dma_start(clo[:, soffs[si]:soffs[si] + Ls],
                                clo_d[:, soffs[si]:soffs[si] + Ls])
        zeros = consts.tile([128, SLOT_CAP], F16)
        nc.vector.memset(zeros[:], 0.0)

        img_ps = imgp.tile([128, 3 * NPC], F32, name="img_ps")

        copy_rot = [0]

        def emit_phase_a(si):
            Ls = sum(slots[si])
            so = soffs[si]
            dps = dmmp.tile([128, SLOT_CAP], F32, tag="dps")
            for h0 in range(0, Ls, 512):
                h1 = min(Ls, h0 + 512)
                nc.tensor.matmul(dps[:, h0:h1], mono[:],
                                 chi[:, so + h0:so + h1],
                                 start=True, stop=False)
                nc.tensor.matmul(dps[:, h0:h1], mono[:],
                                 clo[:, so + h0:so + h1],
                                 start=False, stop=True)
            alpha = apool.tile([128, SLOT_CAP], F16, tag="alpha")
            nc.scalar.activation(alpha[:, :Ls], dps[:, :Ls], AF.Exp,
                                 scale=-1.0)
            mcull = apool.tile([128, SLOT_CAP], F16, tag="mcull")
            nc.gpsimd.tensor_scalar(mcull[:, :Ls], alpha[:, :Ls],
                                    1.0 / 255.0, None, ALU.is_lt)
            na = apool.tile([128, SLOT_CAP], F16, tag="na")
            nc.vector.tensor_scalar(na[:, :Ls], alpha[:, :Ls], -1.0, 1.0,
                                    ALU.mult, ALU.add)
            if flags[si]:
                nc.vector.tensor_scalar(na[:, :Ls], na[:, :Ls], 0.01, None,
                                        ALU.max)
            om = apool.tile([128, SLOT_CAP], F16, tag="om")
            nc.vector.tensor_tensor(om[:, :Ls], na[:, :Ls], mcull[:, :Ls],
                                    ALU.max)
            tbuf = tpool.tile([128, SLOT_CAP], F16, tag="tbuf")
            toff = 0
            for Lp in slots[si]:
                nc.vector.tensor_tensor_scan(
                    tbuf[:, toff:toff + Lp], om[:, toff:toff + Lp],
                    zeros[:, :Lp], 1.0, ALU.mult, ALU.add)
                toff += Lp
            return tbuf

        def emit_phase_b(si, tbuf):
            Ls = sum(slots[si])
            nch = -(-Ls // 128)
            for g0 in range(0, nch, 4):
                g1 = min(nch, g0 + 4)
                tp = tpp.tile([128, 512], F16, tag="tp")
                for k in range(g0, g1):
                    wk = min(128, Ls - k * 128)
                    nc.tensor.transpose(
                        tp[0:wk, (k - g0) * 128:(k - g0) * 128 + 128],
                        tbuf[:, k * 128:k * 128 + wk], ident[:])
                tts = ttspool.tile([128, 512], F16, tag="tts")
                rot = copy_rot[0] % 4
                copy_rot[0] += 1
                width = (g1 - g0 - 1) * 128 + 128
                if rot in (0, 2):
                    nc.vector.tensor_copy(tts[:, :width], tp[:, :width])
                else:
                    nc.scalar.activation(tts[:, :width], tp[:, :width],
                                         AF.Copy)
                for (ck, lo, hi, tj, ov, first, last) in struct["overlaps"][si]:
                    if not (g0 <= ck < g1):
                        continue
                    gidx = struct["gidx"][si][tj]
                    nc.tensor.matmul(
                        img_ps[:, 3 * gidx:3 * gidx + 3],
                        tts[:, (ck - g0) * 128:(ck - g0) * 128 + 128],
                        dcw[:, 3 * ov:3 * ov + 3],
                        start=first, stop=last)
            # per-slot image drain
            glo = struct["gidx"][si][0]
            ghi = struct["gidx"][si][-1] + 1
            ob = outp.tile([128, 3 * NPC], F32, tag="ob")
            nc.vector.tensor_copy(ob[:, 3 * glo:3 * ghi],
                                  img_ps[:, 3 * glo:3 * ghi])
            nc.sync.dma_start(img_d[:, 3 * glo:3 * ghi],
                              ob[:, 3 * glo:3 * ghi])

        prev = None
        for si in range(len(slots)):
            tbuf = emit_phase_a(si)
            if prev is not None:
                emit_phase_b(si - 1, prev)
            prev = tbuf
        emit_phase_b(len(slots) - 1, prev)

    saved = bacc.get_activation_tables
    bacc.get_activation_tables = _only_full_act_set
    try:
        nc.compile()
    finally:
        bacc.get_activation_tables = saved
    return nc


# ------------------------------------------------------------------ driver
def kernel(context_pose, target_poses, target_intrinsics, means1, means2,
           cov1, cov2, sh1, sh2, op1, op2, background_color,
           image_h, image_w):
    f32 = np.float32
    b, v = np.asarray(target_poses).shape[:2]
    assert b == 1 and v == NCAM and int(image_h) == H and int(image_w) == W

    context_pose = np.asarray(context_pose, f32)
    target_poses = np.asarray(target_poses, f32)
    target_intrinsics = np.asarray(target_intrinsics, f32)
    bg = np.asarray(background_color, f32)

    try:
        inv_base = np.linalg.inv(
            context_pose[0].astype(np.float64)).astype(f32)
    except np.linalg.LinAlgError:
        inv_base = np.linalg.pinv(
            context_pose[0].astype(np.float64)).astype(f32)
    d_sh = np.asarray(sh1).shape[-1]
    means = np.stack([np.asarray(means1, f32), np.asarray(means2, f32)],
                     1).reshape(-1, 3)
    covs = np.stack([np.asarray(cov1, f32), np.asarray(cov2, f32)],
                    1).reshape(-1, 3, 3)
    shs = np.stack([np.asarray(sh1, f32), np.asarray(sh2, f32)],
                   1).reshape(-1, 3, d_sh)
    ops = np.stack([np.asarray(op1, f32), np.asarray(op2, f32)],
                   1).reshape(-1)

    row_scale = np.array([1.0 / W, 1.0 / H, 1.0], f32)[:, None]
    cams = []
    for cam in range(NCAM):
        extr = inv_base @ target_poses[0, cam]
        Kn = target_intrinsics[0, cam] * row_scale
        K = np.array([[Kn[0, 0] * W, 0, Kn[0, 2] * W],
                      [0, Kn[1, 1] * H, Kn[1, 2] * H],
                      [0, 0, 1]], f32)
        cams.append(_prep_camera(extr, K, means, covs, shs, ops))
    assert all(c["psd"] for c in cams), "non-PSD conics unsupported in v4"

    # per-tile data
    tiles = []
    for cam in range(NCAM):
        for ty in range(NTY):
            for tx in range(NTX):
                coef, dc, c1, amax = _tile_data(cams[cam], ty, tx, bg)
                tiles.append(dict(cam=cam, ty=ty, tx=tx, coef=coef, dc=dc,
                                  c1=c1, amax=amax, L=coef.shape[1]))

    # snake assignment of size-sorted tiles to cores
    order = sorted(range(NTILE), key=lambda t: -tiles[t]["L"])
    percore = [[] for _ in range(8)]
    for k, t in enumerate(order):
        core = k % 8 if (k // 8) % 2 == 0 else 7 - (k % 8)
        percore[core].append(t)
    for core in range(8):
        percore[core].sort(key=lambda t: -tiles[t]["L"])

    # per-rank padded lengths (identical across cores)
    lpad = [max(1, max(tiles[percore[c][r]]["L"] for c in range(8)))
            for r in range(NPC)]

    # first-fit-decreasing into slots of <= SLOT_CAP columns
    slots_ranks, slots_len = [], []
    for r in range(NPC):
        placed = False
        for si in range(len(slots_ranks)):
            if slots_len[si] + lpad[r] <= SLOT_CAP:
                slots_ranks[si].append(r)
                slots_len[si] += lpad[r]
                placed = True
                break
        if not placed:
            slots_ranks.append([r])
            slots_len.append(lpad[r])
    slots = [[lpad[r] for r in ranks] for ranks in slots_ranks]

    # clamp flags per slot (any core instance with alpha near/above 0.99)
    flags = []
    for ranks in slots_ranks:
        mx = max(tiles[percore[c][r]]["amax"]
                 for r in ranks for c in range(8))
        flags.append(bool(mx > 0.9895))

    # chunk overlap structure + global tile-slot indices
    overlaps, gidx = [], []
    g = 0
    ov = 0
    for si, s in enumerate(slots):
        gidx.append(list(range(g, g + len(s))))
        g += len(s)
        ovs = []
        toff = 0
        for tj, Lp in enumerate(s):
            lo, hi = toff, toff + Lp
            ck0, ck1 = lo // 128, (hi - 1) // 128
            for ck in range(ck0, ck1 + 1):
                a = max(lo, ck * 128)
                bnd = min(hi, ck * 128 + 128)
                ovs.append((ck, a, bnd, tj, ov, ck == ck0, ck == ck1))
                ov += 1
            toff += Lp
        overlaps.append(ovs)
    novl = ov
    struct = dict(slots=slots, flags=tuple(flags), novl=novl,
                  overlaps=overlaps, gidx=gidx)

    key = (tuple(tuple(s) for s in slots), tuple(flags), novl)
    if key not in _NC_CACHE:
        _NC_CACHE[key] = _build_nc(struct)
    nc = _NC_CACHE[key]

    # constants
    f16 = np.float16
    cvec = np.arange(TC, dtype=f32) - (TC / 2.0 - 0.5)
    rvec = np.arange(TR, dtype=f32) - (TR / 2.0 - 0.5)
    yyt, xxt = np.meshgrid(rvec, cvec, indexing="ij")
    xt, yt = xxt.reshape(-1), yyt.reshape(-1)      # [128] tile-local coords
    mono = np.stack([xt * xt, xt * yt, yt * yt, xt, yt,
                     np.ones(128, f32)], 0)
    mono16 = mono.astype(f16)
    assert np.all(mono16.astype(f32) == mono)
    ident = np.eye(128, dtype=f16)

    SL = sum(sum(s) for s in slots)
    in_maps = []
    for core in range(8):
        chi = np.zeros((6, SL), f32)
        chi[5, :] = PAD_F
        dcw = np.zeros((128, 3 * novl), f16)
        off = 0
        for si, s in enumerate(slots):
            toff = 0
            for tj, Lp in enumerate(s):
                t = tiles[percore[core][slots_ranks[si][tj]]]
                L = t["L"]
                chi[:, off + toff:off + toff + L] = t["coef"]
                toff += Lp
            for (ck, lo, hi, tj, ovi, first, last) in overlaps[si]:
                t = tiles[percore[core][slots_ranks[si][tj]]]
                L = t["L"]
                tstart = sum(s[:tj])
                r0 = lo - ck * 128
                for j in range(lo, hi):
                    gj = j - tstart
                    if gj < L:
                        dcw[r0 + (j - lo), 3 * ovi:3 * ovi + 3] = t["dc"][gj]
            off += sum(s)
        chi16 = chi.astype(f16)
        clo16 = (chi - chi16.astype(f32)).astype(f16)
        in_maps.append({"mono": mono16, "ident": ident, "chi": chi16,
                        "clo": clo16, "dcw": dcw})

    trace = os.environ.get("SPLAT_TRACE", "0") == "1"
    res = run_bass_kernel_spmd(nc, in_maps, core_ids=list(range(8)),
                               trace=trace,
                               trace_cores=list(range(8)) if trace else None)
    global _LAST_EXEC_NS, _LAST_RESULTS
    _LAST_EXEC_NS = res.exec_time_ns
    _LAST_RESULTS = res

    out = np.zeros((1, NCAM, 3, H, W), f32)
    for core in range(8):
        img = res.results[core]["img"]          # [128, 3*NPC]
        for si, ranks in enumerate(slots_ranks):
            for tj, r in enumerate(ranks):
                t = tiles[percore[core][r]]
                gg = gidx[si][tj]
                piece = img[:, 3 * gg:3 * gg + 3].T.reshape(3, TR, TC)
                cam, ty, tx = t["cam"], t["ty"], t["tx"]
                out[0, cam, :, ty * TR:(ty + 1) * TR,
                    tx * TC:(tx + 1) * TC] = piece + t["c1"][:, None, None]
    return out


# revision 5
# speedup vs baseline: 4.4528x; 1.1052x over previous
"""Trainium2 Bass kernel for DecoderSplattingCUDA (EWA Gaussian splatting).

Contract: kernel(**inputs) takes the FULL inputs of reference.setup_inputs()
and returns the FULL [b, v, 3, H, W] image, computed on 8 NeuronCores.

Layout (v4): PIXELS on partitions, gaussians along the free axis.
The image is cut into 256 tiles of 8x16 = 128 pixels (one partition per
pixel).  Per tile the host culls gaussians by their exact peak alpha and
emits, per survivor, the 6 coefficients of the screen-space quadratic
  D(x,y) = A x~^2 + B x~y~ + C y~^2 + Dx x~ + Ey y~ + F   (tile-centered)
with alpha = exp(-D) already folding in opacity (F includes -log(op)).

Device per (tile batch = slot of <=1024 survivor columns):
  D     = matmul(mono[6,128]^T, coeff[6,L])   PE, fp16 hi+lo (exact-ish)
  alpha = Exp(-D)                              ACT, psum -> sbuf fp16
  mcull = alpha < 1/255                        Pool
  na    = 1 - alpha  (max 0.01 if clamp slot)  DVE dual-op
  om    = max(na, mcull)                       DVE   (culled -> om = 1)
  T     = tensor_tensor_scan(om, mult)         DVE, per tile, init 1.0
  Tt    = PE transpose per 128-col chunk -> psum fp16 -> sbuf
  img^T[128px,3] += Tt_chunk^T @ dc[128,3]     PE, accumulated per tile
Host adds the summation-by-parts constant c1 per tile and reassembles.
T_g = prod_{i<=g}(1-alpha_i) exactly matches the reference compositing
order (depth-sorted survivor lists), with img = c1 + sum_g T_g dc_g.
"""
import os
import sys

sys.path.insert(0, "/opt/trn_rl_repo/concourse")

from contextlib import ExitStack

import numpy as np

import concourse.bacc as bacc
import concourse.tile as tile
from concourse import mybir
from concourse.bass_utils import run_bass_kernel_spmd
from concourse.hw_specs import get_activation_tables

F32 = mybir.dt.float32
F16 = mybir.dt.float16
AF = mybir.ActivationFunctionType
ALU = mybir.AluOpType

C0 = 0.28209479177387814
C1 = 0.4886025119029199
NEAR, FAR = 0.1, 1000.0
LN255 = float(np.float32(np.log(np.float32(255.0))))
NEG_BIG = -200.0

H = W = 128
NCAM = 2
TR, TC = 8, 16                  # tile shape (rows x cols) = 128 px
NTY, NTX = H // TR, W // TC     # 16 x 8 tiles per camera
NTILE = NCAM * NTY * NTX        # 256
NPC = NTILE // 8                # tiles per core (32)
SLOT_CAP = 512                  # max survivor columns per slot (psum bank)
PAD_F = 30000.0                 # padding column: D = PAD_F -> alpha = 0

_NC_CACHE = {}
_LAST_EXEC_NS = None
_LAST_RESULTS = None


def _only_full_act_set(arch):
    full = get_activation_tables(arch)
    keep = "natural_log_exp_and_others"
    return {name: (fns if name == keep else set()) for name, fns in full.items()}


# ---------------------------------------------------------------- host prep
def _prep_camera(extr, K, means, cov, sh, op):
    """Per-gaussian camera math (numpy f32), depth-sorted."""
    f32 = np.float32
    extr = extr.astype(f32)
    try:
        w2c = np.linalg.inv(extr.astype(np.float64)).astype(f32)
    except np.linalg.LinAlgError:
        w2c = np.linalg.pinv(extr.astype(np.float64)).astype(f32)
    R, t = w2c[:3, :3], w2c[:3, 3]
    p = means @ R.T + t
    x, y, z = p[:, 0], p[:, 1], p[:, 2]
    zc = np.maximum(z, f32(1e-6))
    fx, fy = K[0, 0], K[1, 1]
    cx, cy = K[0, 2], K[1, 2]
    u = fx * x / zc + cx
    v = fy * y / zc + cy
    cov_c = np.einsum("ij,gjk,lk->gil", R, cov, R)
    zero = np.zeros_like(zc)
    J = np.stack([np.stack([fx / zc, zero, -fx * x / (zc * zc)], -1),
                  np.stack([zero, fy / zc, -fy * y / (zc * zc)], -1)], -2)
    cov2d = np.einsum("gij,gjk,glk->gil", J, cov_c, J)
    a = cov2d[:, 0, 0] + f32(0.3)
    bb = cov2d[:, 0, 1]
    c = cov2d[:, 1, 1] + f32(0.3)
    det = np.maximum(a * c - bb * bb, f32(1e-12))
    ia, ib, ic = c / det, -bb / det, a / det
    d = means - extr[:3, 3]
    d = d / np.linalg.norm(d, axis=-1, keepdims=True)
    col = C0 * sh[:, :, 0]
    if sh.shape[-1] >= 4:
        col = (col - C1 * d[:, 1:2] * sh[:, :, 1]
               + C1 * d[:, 2:3] * sh[:, :, 2]
               - C1 * d[:, 0:1] * sh[:, :, 3])
    col = np.maximum(col + f32(0.5), f32(0.0)).astype(f32)

    valid = (z > f32(NEAR)) & (z < f32(FAR))
    op_eff = np.where(valid, op, f32(0.0))
    order = np.argsort(z, kind="stable")
    u, v, ia, ib, ic, op_eff = (arr[order] for arr in
                                (u, v, ia, ib, ic, op_eff))
    col = col[order]

    psd_g = (ia > 0) & (ic - np.where(ia != 0, ib * ib / ia, 0.0) > 0)
    with np.errstate(divide="ignore", invalid="ignore"):
        r = np.where(ia != 0, ib / ia, f32(0.0)).astype(f32)
        eta = ic - np.where(ia != 0, ib * ib / ia, f32(0.0))
        gamma2 = (np.abs(ia) * f32(0.5)).astype(f32)
        delta2 = (np.abs(eta) * f32(0.5)).astype(f32)
        logop = np.where(op_eff > 0, np.log(np.maximum(op_eff, f32(1e-30))),
                         f32(NEG_BIG))
    logop = np.maximum(logop, f32(NEG_BIG)).astype(f32)
    return dict(u=u.astype(f32), v=v.astype(f32), r=r, gamma2=gamma2,
                delta2=delta2, logop=logop, col=col,
                psd=bool(np.all(psd_g)))


def _tile_data(cp, ty, tx, bg):
    """Exact cull for tile (ty, tx); returns per-survivor coeffs, dc, c1,
    and the max unclamped alpha (for the 0.99-clamp flag)."""
    f32 = np.float32
    r0, c0 = ty * TR, tx * TC
    u, v, r = cp["u"], cp["v"], cp["r"]
    g2, d2, logop = cp["gamma2"], cp["delta2"], cp["logop"]
    # conservative candidate box test
    ylo, yhi = f32(r0 + 0.5), f32(r0 + TR - 0.5)
    xlo, xhi = f32(c0 + 0.5), f32(c0 + TC - 0.5)
    dymin = np.maximum(0.0, np.maximum(ylo - v, v - yhi)).astype(f32)
    dy_a, dy_b = ylo - v, yhi - v
    x0_a, x0_b = u - r * dy_a, u - r * dy_b
    x0_lo = np.minimum(x0_a, x0_b)
    x0_hi = np.maximum(x0_a, x0_b)
    dxmin = np.maximum(0.0, np.maximum(x0_lo - xhi, xlo - x0_hi)).astype(f32)
    q = d2 * dymin ** 2 + g2 * dxmin ** 2
    cand = np.nonzero(q <= logop + f32(LN255 + 0.02))[0]
    if len(cand) == 0:
        return (np.zeros((6, 0), f32), np.zeros((0, 3), f32),
                bg.astype(f32).copy(), 0.0)
    # exact alpha over the 128 pixels for candidates
    xs = np.arange(c0, c0 + TC, dtype=f32) + 0.5
    ys = np.arange(r0, r0 + TR, dtype=f32) + 0.5
    yy, xx = np.meshgrid(ys, xs, indexing="ij")
    xx, yy = xx.reshape(-1), yy.reshape(-1)
    gu, gv, gr = u[cand, None], v[cand, None], r[cand, None]
    gg2, gd2, glo = g2[cand, None], d2[cand, None], logop[cand, None]
    dx = xx[None, :] - gu
    dyv = yy[None, :] - gv
    D = gg2 * (dx + gr * dyv) ** 2 + gd2 * dyv ** 2 - glo
    amax = np.exp(-np.maximum(D.min(axis=1), 0.0))
    keep = amax >= f32(1.0 / 255.0) - f32(1e-6)
    idx = cand[keep]
    if len(idx) == 0:
        return (np.zeros((6, 0), f32), np.zeros((0, 3), f32),
                bg.astype(f32).copy(), 0.0)
    # tile-centered quadratic coefficients
    x0f, y0f = f32(c0 + TC / 2.0), f32(r0 + TR / 2.0)
    ut, vt = u[idx] - x0f, v[idx] - y0f
    rr, gg, dd, lo = r[idx], g2[idx], d2[idx], logop[idx]
    st = ut + rr * vt
    coef = np.stack([gg,
                     2 * gg * rr,
                     gg * rr * rr + dd,
                     -2 * gg * st,
                     -2 * gg * rr * st - 2 * dd * vt,
                     gg * st * st + dd * vt * vt - lo], 0).astype(f32)
    col = cp["col"][idx]
    n = len(idx)
    dc = np.zeros((n, 3), f32)
    dc[:-1] = col[1:] - col[:-1]
    dc[-1] = bg - col[-1]
    return coef, dc, col[0].copy(), float(amax[keep].max())


# ------------------------------------------------------------- bass program
def _build_nc(struct):
    """struct: dict with
      slots: list of slots; each slot = list of padded tile lengths
      flags: per-slot bool (apply 0.99 clamp)
      novl:  total number of (chunk, tile) overlap color matmuls
      overlaps: per slot: list of (chunk_local_idx, col_lo, col_hi,
                 tile_idx_in_slot, ov_idx, is_first, is_last)
    """
    slots = struct["slots"]
    flags = struct["flags"]
    novl = struct["novl"]
    SL = sum(sum(s) for s in slots)
    nc = bacc.Bacc(None, target_bir_lowering=False)

    mono_d = nc.dram_tensor("mono", [6, 128], F16, kind="ExternalInput")
    ident_d = nc.dram_tensor("ident", [128, 128], F16, kind="ExternalInput")
    chi_d = nc.dram_tensor("chi", [6, SL], F16, kind="ExternalInput")
    clo_d = nc.dram_tensor("clo", [6, SL], F16, kind="ExternalInput")
    dcw_d = nc.dram_tensor("dcw", [128, 3 * novl], F16, kind="ExternalInput")
    img_d = nc.dram_tensor("img", [128, 3 * NPC], F32, kind="ExternalOutput")

    with tile.TileContext(nc) as tc, ExitStack() as ctx:
        consts = ctx.enter_context(tc.tile_pool(name="consts", bufs=1))
        apool = ctx.enter_context(tc.tile_pool(name="apool", bufs=2))
        tpool = ctx.enter_context(tc.tile_pool(name="tpool", bufs=2))
        ttspool = ctx.enter_context(tc.tile_pool(name="ttspool", bufs=3))
        outp = ctx.enter_context(tc.tile_pool(name="outp", bufs=2))
        dmmp = ctx.enter_context(tc.tile_pool(name="dmmp", bufs=2,
                                              space="PSUM"))
        tpp = ctx.enter_context(tc.tile_pool(name="tpp", bufs=3,
                                             space="PSUM"))
        imgp = ctx.enter_context(tc.tile_pool(name="imgp", bufs=1,
                                              space="PSUM"))

        mono = consts.tile([6, 128], F16)
        ident = consts.tile([128, 128], F16)
        chi = consts.tile([6, SL], F16)
        clo = consts.tile([6, SL], F16)
        dcw = consts.tile([128, 3 * novl], F16)
        soffs = []
        off = 0
        for s in slots:
            soffs.append(off)
            off += sum(s)
        # input DMAs on the otherwise-idle SP queue; ordered so slot 0's
        # operands land first
        half = soffs[len(slots) // 2]
        nc.sync.dma_start(mono[:], mono_d[:])
        nc.sync.dma_start(chi[:, :half], chi_d[:, :half])
        nc.sync.dma_start(clo[:, :half], clo_d[:, :half])
        nc.sync.dma_start(ident[:], ident_d[:])
        nc.sync.dma_start(dcw[:], dcw_d[:])
        nc.sync.dma_start(chi[:, half:], chi_d[:, half:])
        nc.sync.dma_start(clo[:, half:], clo_d[:, half:])
        zeros = consts.tile([128, SLOT_CAP], F16)
        nc.gpsimd.memset(zeros[:], 0.0)

        img_ps = imgp.tile([128, 3 * NPC], F32, name="img_ps")

        copy_rot = [0]
        tbufs = {}

        def emit_dmm(si):
            Ls = sum(slots[si])
            so = soffs[si]
            dps = dmmp.tile([128, SLOT_CAP], F32, tag="dps")
            nc.tensor.matmul(dps[:, :Ls], mono[:], chi[:, so:so + Ls],
                             start=True, stop=False)
            nc.tensor.matmul(dps[:, :Ls], mono[:], clo[:, so:so + Ls],
                             start=False, stop=True)
            tbufs[si] = dict(dps=dps)

        def emit_transposes(si):
            st = tbufs[si]
            Ls = sum(slots[si])
            nch = -(-Ls // 128)
            tp = tpp.tile([128, 512], F16, tag="tp")
            for k in range(nch):
                wk = min(128, Ls - k * 128)
                nc.tensor.transpose(tp[0:wk, k * 128:k * 128 + 128],
                                    st["tbuf"][:, k * 128:k * 128 + wk],
                                    ident[:])
            st["tp"] = tp
            st["nch"] = nch

        def emit_copy(si):
            st = tbufs[si]
            width = st["nch"] * 128
            tts = ttspool.tile([128, 512], F16, tag="tts")
            if copy_rot[0] % 2 == 0:
                nc.vector.tensor_copy(tts[:, :width], st["tp"][:, :width])
            else:
                nc.scalar.activation(tts[:, :width], st["tp"][:, :width],
                                     AF.Copy)
            copy_rot[0] += 1
            st["tts"] = tts

        def emit_exp_mask(si):
            st = tbufs[si]
            Ls = sum(slots[si])
            alpha = apool.tile([128, SLOT_CAP], F16, tag="alpha")
            nc.scalar.activation(alpha[:, :Ls], st["dps"][:, :Ls], AF.Exp,
                                 scale=-1.0)
            mcull = apool.tile([128, SLOT_CAP], F16, tag="mcull")
            nc.vector.tensor_scalar(mcull[:, :Ls], alpha[:, :Ls],
                                    1.0 / 255.0, None, ALU.is_lt)
            na = apool.tile([128, SLOT_CAP], F16, tag="na")
            if flags[si]:
                nc.vector.tensor_scalar(na[:, :Ls], alpha[:, :Ls], -1.0,
                                        1.0, ALU.mult, ALU.add)
                nc.vector.tensor_scalar(na[:, :Ls], na[:, :Ls], 0.01, None,
                                        ALU.max)
            else:
                nc.vector.tensor_scalar(na[:, :Ls], alpha[:, :Ls], -1.0,
                                        1.0, ALU.mult, ALU.add)
            om = apool.tile([128, SLOT_CAP], F16, tag="om")
            nc.vector.tensor_tensor(om[:, :Ls], na[:, :Ls], mcull[:, :Ls],
                                    ALU.max)
            st["om"] = om

        def emit_scans(si):
            st = tbufs[si]
            tbuf = tpool.tile([128, SLOT_CAP], F16, tag="tbuf")
            toff = 0
            for Lp in slots[si]:
                nc.vector.tensor_tensor_scan(
                    tbuf[:, toff:toff + Lp], st["om"][:, toff:toff + Lp],
                    zeros[:, :Lp], 1.0, ALU.mult, ALU.add)
                toff += Lp
            st["tbuf"] = tbuf

        def emit_colors_drain(si):
            st = tbufs[si]
            tts = st["tts"]
            for (ck, lo, hi, tj, ov, first, last) in struct["overlaps"][si]:
                gidx = struct["gidx"][si][tj]
                nc.tensor.matmul(
                    img_ps[:, 3 * gidx:3 * gidx + 3],
                    tts[:, ck * 128:ck * 128 + 128],
                    dcw[:, 3 * ov:3 * ov + 3],
                    start=first, stop=last)
            glo = struct["gidx"][si][0]
            ghi = struct["gidx"][si][-1] + 1
            ob = outp.tile([128, 3 * NPC], F32, tag="ob")
            nc.vector.tensor_copy(ob[:, 3 * glo:3 * ghi],
                                  img_ps[:, 3 * glo:3 * ghi])
            nc.sync.dma_start(img_d[:, 3 * glo:3 * ghi],
                              ob[:, 3 * glo:3 * ghi])
            del st["dps"]

        n = len(slots)
        for si in range(n):
            emit_dmm(si)
            if si > 0:
                emit_transposes(si - 1)
                emit_copy(si - 1)
            emit_exp_mask(si)
            if si > 0:
                emit_colors_drain(si - 1)
            emit_scans(si)
        emit_transposes(n - 1)
        emit_copy(n - 1)
        emit_colors_drain(n - 1)

    saved = bacc.get_activation_tables
    bacc.get_activation_tables = _only_full_act_set
    try:
        nc.compile()
    finally:
        bacc.get_activation_tables = saved
    return nc


# ------------------------------------------------------------------ driver
def kernel(context_pose, target_poses, target_intrinsics, means1, means2,
           cov1, cov2, sh1, sh2, op1, op2, background_color,
           image_h, image_w):
    f32 = np.float32
    b, v = np.asarray(target_poses).shape[:2]
    assert b == 1 and v == NCAM and int(image_h) == H and int(image_w) == W

    context_pose = np.asarray(context_pose, f32)
    target_poses = np.asarray(target_poses, f32)
    target_intrinsics = np.asarray(target_intrinsics, f32)
    bg = np.asarray(background_color, f32)

    try:
        inv_base = np.linalg.inv(
            context_pose[0].astype(np.float64)).astype(f32)
    except np.linalg.LinAlgError:
        inv_base = np.linalg.pinv(
            context_pose[0].astype(np.float64)).astype(f32)
    d_sh = np.asarray(sh1).shape[-1]
    means = np.stack([np.asarray(means1, f32), np.asarray(means2, f32)],
                     1).reshape(-1, 3)
    covs = np.stack([np.asarray(cov1, f32), np.asarray(cov2, f32)],
                    1).reshape(-1, 3, 3)
    shs = np.stack([np.asarray(sh1, f32), np.asarray(sh2, f32)],
                   1).reshape(-1, 3, d_sh)
    ops = np.stack([np.asarray(op1, f32), np.asarray(op2, f32)],
                   1).reshape(-1)

    row_scale = np.array([1.0 / W, 1.0 / H, 1.0], f32)[:, None]
    cams = []
    for cam in range(NCAM):
        extr = inv_base @ target_poses[0, cam]
        Kn = target_intrinsics[0, cam] * row_scale
        K = np.array([[Kn[0, 0] * W, 0, Kn[0, 2] * W],
                      [0, Kn[1, 1] * H, Kn[1, 2] * H],
                      [0, 0, 1]], f32)
        cams.append(_prep_camera(extr, K, means, covs, shs, ops))
    assert all(c["psd"] for c in cams), "non-PSD conics unsupported in v4"

    # per-tile data
    tiles = []
    for cam in range(NCAM):
        for ty in range(NTY):
            for tx in range(NTX):
                coef, dc, c1, amax = _tile_data(cams[cam], ty, tx, bg)
                tiles.append(dict(cam=cam, ty=ty, tx=tx, coef=coef, dc=dc,
                                  c1=c1, amax=amax, L=coef.shape[1]))

    # snake assignment of size-sorted tiles to cores
    order = sorted(range(NTILE), key=lambda t: -tiles[t]["L"])
    percore = [[] for _ in range(8)]
    for k, t in enumerate(order):
        core = k % 8 if (k // 8) % 2 == 0 else 7 - (k % 8)
        percore[core].append(t)
    for core in range(8):
        percore[core].sort(key=lambda t: -tiles[t]["L"])

    # per-rank padded lengths (identical across cores)
    lpad = [max(1, max(tiles[percore[c][r]]["L"] for c in range(8)))
            for r in range(NPC)]

    # first-fit-decreasing into slots of <= SLOT_CAP columns
    slots_ranks, slots_len = [], []
    for r in range(NPC):
        placed = False
        for si in range(len(slots_ranks)):
            if slots_len[si] + lpad[r] <= SLOT_CAP:
                slots_ranks[si].append(r)
                slots_len[si] += lpad[r]
                placed = True
                break
        if not placed:
            slots_ranks.append([r])
            slots_len.append(lpad[r])
    slots = [[lpad[r] for r in ranks] for ranks in slots_ranks]

    # clamp flags per slot (any core instance with alpha near/above 0.99)
    flags = []
    for ranks in slots_ranks:
        mx = max(tiles[percore[c][r]]["amax"]
                 for r in ranks for c in range(8))
        flags.append(bool(mx > 0.9895))

    # chunk overlap structure + global tile-slot indices
    overlaps, gidx = [], []
    g = 0
    ov = 0
    for si, s in enumerate(slots):
        gidx.append(list(range(g, g + len(s))))
        g += len(s)
        ovs = []
        toff = 0
        for tj, Lp in enumerate(s):
            lo, hi = toff, toff + Lp
            ck0, ck1 = lo // 128, (hi - 1) // 128
            for ck in range(ck0, ck1 + 1):
                a = max(lo, ck * 128)
                bnd = min(hi, ck * 128 + 128)
                ovs.append((ck, a, bnd, tj, ov, ck == ck0, ck == ck1))
                ov += 1
            toff += Lp
        overlaps.append(ovs)
    novl = ov
    struct = dict(slots=slots, flags=tuple(flags), novl=novl,
                  overlaps=overlaps, gidx=gidx)

    key = (tuple(tuple(s) for s in slots), tuple(flags), novl)
    if key not in _NC_CACHE:
        _NC_CACHE[key] = _build_nc(struct)
    nc = _NC_CACHE[key]

    # constants
    f16 = np.float16
    cvec = np.arange(TC, dtype=f32) - (TC / 2.0 - 0.5)
    rvec = np.arange(TR, dtype=f32) - (TR / 2.0 - 0.5)
    yyt, xxt = np.meshgrid(rvec, cvec, indexing="ij")
    xt, yt = xxt.reshape(-1), yyt.reshape(-1)      # [128] tile-local coords
    mono = np.stack([xt * xt, xt * yt, yt * yt, xt, yt,
                     np.ones(128, f32)], 0)
    mono16 = mono.astype(f16)
    assert np.all(mono16.astype(f32) == mono)
    ident = np.eye(128, dtype=f16)

    SL = sum(sum(s) for s in slots)
    in_maps = []
    for core in range(8):
        chi = np.zeros((6, SL), f32)
        chi[5, :] = PAD_F
        dcw = np.zeros((128, 3 * novl), f16)
        off = 0
        for si, s in enumerate(slots):
            toff = 0
            for tj, Lp in enumerate(s):
                t = tiles[percore[core][slots_ranks[si][tj]]]
                L = t["L"]
                chi[:, off + toff:off + toff + L] = t["coef"]
                toff += Lp
            for (ck, lo, hi, tj, ovi, first, last) in overlaps[si]:
                t = tiles[percore[core][slots_ranks[si][tj]]]
                L = t["L"]
                tstart = sum(s[:tj])
                r0 = lo - ck * 128
                for j in range(lo, hi):
                    gj = j - tstart
                    if gj < L:
                        dcw[r0 + (j - lo), 3 * ovi:3 * ovi + 3] = t["dc"][gj]
            off += sum(s)
        chi16 = chi.astype(f16)
        clo16 = (chi - chi16.astype(f32)).astype(f16)
        in_maps.append({"mono": mono16, "ident": ident, "chi": chi16,
                        "clo": clo16, "dcw": dcw})

    trace = os.environ.get("SPLAT_TRACE", "0") == "1"
    res = run_bass_kernel_spmd(nc, in_maps, core_ids=list(range(8)),
                               trace=trace,
                               trace_cores=list(range(8)) if trace else None)
    global _LAST_EXEC_NS, _LAST_RESULTS
    _LAST_EXEC_NS = res.exec_time_ns
    _LAST_RESULTS = res

    out = np.zeros((1, NCAM, 3, H, W), f32)
    for core in range(8):
        img = res.results[core]["img"]          # [128, 3*NPC]
        for si, ranks in enumerate(slots_ranks):
            for tj, r in enumerate(ranks):
                t = tiles[percore[core][r]]
                gg = gidx[si][tj]
                piece = img[:, 3 * gg:3 * gg + 3].T.reshape(3, TR, TC)
                cam, ty, tx = t["cam"], t["ty"], t["tx"]
                out[0, cam, :, ty * TR:(ty + 1) * TR,
                    tx * TC:(tx + 1) * TC] = piece + t["c1"][:, None, None]
    return out


# revision 17
# speedup vs baseline: 5.1794x; 1.1632x over previous
"""Trainium2 Bass kernel for DecoderSplattingCUDA (EWA Gaussian splatting).

Contract: kernel(**inputs) takes the FULL inputs of reference.setup_inputs()
and returns the FULL [b, v, 3, H, W] image, computed on 8 NeuronCores.

Layout (v4): PIXELS on partitions, gaussians along the free axis.
The image is cut into 256 tiles of 8x16 = 128 pixels (one partition per
pixel).  Per tile the host culls gaussians by their exact peak alpha and
emits, per survivor, the 6 coefficients of the screen-space quadratic
  D(x,y) = A x~^2 + B x~y~ + C y~^2 + Dx x~ + Ey y~ + F   (tile-centered)
with alpha = exp(-D) already folding in opacity (F includes -log(op)).

Device per (tile batch = slot of <=1024 survivor columns):
  D     = matmul(mono[6,128]^T, coeff[6,L])   PE, fp16 hi+lo (exact-ish)
  alpha = Exp(-D)                              ACT, psum -> sbuf fp16
  mcull = alpha < 1/255                        Pool
  na    = 1 - alpha  (max 0.01 if clamp slot)  DVE dual-op
  om    = max(na, mcull)                       DVE   (culled -> om = 1)
  T     = tensor_tensor_scan(om, mult)         DVE, per tile, init 1.0
  Tt    = PE transpose per 128-col chunk -> psum fp16 -> sbuf
  img^T[128px,3] += Tt_chunk^T @ dc[128,3]     PE, accumulated per tile
Host adds the summation-by-parts constant c1 per tile and reassembles.
T_g = prod_{i<=g}(1-alpha_i) exactly matches the reference compositing
order (depth-sorted survivor lists), with img = c1 + sum_g T_g dc_g.
"""
import os
import sys

sys.path.insert(0, "/opt/trn_rl_repo/concourse")

from contextlib import ExitStack

import numpy as np

import concourse.bacc as bacc
import concourse.tile as tile
from concourse import mybir
from concourse.bass_utils import run_bass_kernel_spmd
from concourse.hw_specs import get_activation_tables

F32 = mybir.dt.float32
F16 = mybir.dt.float16
AF = mybir.ActivationFunctionType
ALU = mybir.AluOpType

C0 = 0.28209479177387814
C1 = 0.4886025119029199
NEAR, FAR = 0.1, 1000.0
LN255 = float(np.float32(np.log(np.float32(255.0))))
NEG_BIG = -200.0

H = W = 128
NCAM = 2
TR, TC = 8, 16                  # tile shape (rows x cols) = 128 px
NTY, NTX = H // TR, W // TC     # 16 x 8 tiles per camera
NTILE = NCAM * NTY * NTX        # 256
NPC = NTILE // 8                # tiles per core (32)
SLOT_CAP = 512                  # max survivor columns per slot (psum bank)
PAD_F = 30000.0                 # padding column: D = PAD_F -> alpha = 0

_NC_CACHE = {}
_LAST_EXEC_NS = None
_LAST_RESULTS = None


def _only_full_act_set(arch):
    full = get_activation_tables(arch)
    keep = "natural_log_exp_and_others"
    return {name: (fns if name == keep else set()) for name, fns in full.items()}


# ---------------------------------------------------------------- host prep
def _prep_camera(extr, K, means, cov, sh, op):
    """Per-gaussian camera math (numpy f32), depth-sorted."""
    f32 = np.float32
    extr = extr.astype(f32)
    try:
        w2c = np.linalg.inv(extr.astype(np.float64)).astype(f32)
    except np.linalg.LinAlgError:
        w2c = np.linalg.pinv(extr.astype(np.float64)).astype(f32)
    R, t = w2c[:3, :3], w2c[:3, 3]
    p = means @ R.T + t
    x, y, z = p[:, 0], p[:, 1], p[:, 2]
    zc = np.maximum(z, f32(1e-6))
    fx, fy = K[0, 0], K[1, 1]
    cx, cy = K[0, 2], K[1, 2]
    u = fx * x / zc + cx
    v = fy * y / zc + cy
    cov_c = np.einsum("ij,gjk,lk->gil", R, cov, R)
    zero = np.zeros_like(zc)
    J = np.stack([np.stack([fx / zc, zero, -fx * x / (zc * zc)], -1),
                  np.stack([zero, fy / zc, -fy * y / (zc * zc)], -1)], -2)
    cov2d = np.einsum("gij,gjk,glk->gil", J, cov_c, J)
    a = cov2d[:, 0, 0] + f32(0.3)
    bb = cov2d[:, 0, 1]
    c = cov2d[:, 1, 1] + f32(0.3)
    det = np.maximum(a * c - bb * bb, f32(1e-12))
    ia, ib, ic = c / det, -bb / det, a / det
    d = means - extr[:3, 3]
    d = d / np.linalg.norm(d, axis=-1, keepdims=True)
    col = C0 * sh[:, :, 0]
    if sh.shape[-1] >= 4:
        col = (col - C1 * d[:, 1:2] * sh[:, :, 1]
               + C1 * d[:, 2:3] * sh[:, :, 2]
               - C1 * d[:, 0:1] * sh[:, :, 3])
    col = np.maximum(col + f32(0.5), f32(0.0)).astype(f32)

    valid = (z > f32(NEAR)) & (z < f32(FAR))
    op_eff = np.where(valid, op, f32(0.0))
    order = np.argsort(z, kind="stable")
    u, v, ia, ib, ic, op_eff = (arr[order] for arr in
                                (u, v, ia, ib, ic, op_eff))
    col = col[order]

    psd_g = (ia > 0) & (ic - np.where(ia != 0, ib * ib / ia, 0.0) > 0)
    with np.errstate(divide="ignore", invalid="ignore"):
        r = np.where(ia != 0, ib / ia, f32(0.0)).astype(f32)
        eta = ic - np.where(ia != 0, ib * ib / ia, f32(0.0))
        gamma2 = (np.abs(ia) * f32(0.5)).astype(f32)
        delta2 = (np.abs(eta) * f32(0.5)).astype(f32)
        logop = np.where(op_eff > 0, np.log(np.maximum(op_eff, f32(1e-30))),
                         f32(NEG_BIG))
    logop = np.maximum(logop, f32(NEG_BIG)).astype(f32)
    return dict(u=u.astype(f32), v=v.astype(f32), r=r, gamma2=gamma2,
                delta2=delta2, logop=logop, col=col,
                psd=bool(np.all(psd_g)))


def _tile_data(cp, ty, tx, bg):
    """Exact cull for tile (ty, tx); returns per-survivor coeffs, dc, c1,
    and the max unclamped alpha (for the 0.99-clamp flag)."""
    f32 = np.float32
    r0, c0 = ty * TR, tx * TC
    u, v, r = cp["u"], cp["v"], cp["r"]
    g2, d2, logop = cp["gamma2"], cp["delta2"], cp["logop"]
    # conservative candidate box test
    ylo, yhi = f32(r0 + 0.5), f32(r0 + TR - 0.5)
    xlo, xhi = f32(c0 + 0.5), f32(c0 + TC - 0.5)
    dymin = np.maximum(0.0, np.maximum(ylo - v, v - yhi)).astype(f32)
    dy_a, dy_b = ylo - v, yhi - v
    x0_a, x0_b = u - r * dy_a, u - r * dy_b
    x0_lo = np.minimum(x0_a, x0_b)
    x0_hi = np.maximum(x0_a, x0_b)
    dxmin = np.maximum(0.0, np.maximum(x0_lo - xhi, xlo - x0_hi)).astype(f32)
    q = d2 * dymin ** 2 + g2 * dxmin ** 2
    cand = np.nonzero(q <= logop + f32(LN255 + 0.02))[0]
    if len(cand) == 0:
        return (np.zeros((6, 0), f32), np.zeros((0, 3), f32),
                bg.astype(f32).copy(), 0.0)
    # exact alpha over the 128 pixels for candidates
    xs = np.arange(c0, c0 + TC, dtype=f32) + 0.5
    ys = np.arange(r0, r0 + TR, dtype=f32) + 0.5
    yy, xx = np.meshgrid(ys, xs, indexing="ij")
    xx, yy = xx.reshape(-1), yy.reshape(-1)
    gu, gv, gr = u[cand, None], v[cand, None], r[cand, None]
    gg2, gd2, glo = g2[cand, None], d2[cand, None], logop[cand, None]
    dx = xx[None, :] - gu
    dyv = yy[None, :] - gv
    D = gg2 * (dx + gr * dyv) ** 2 + gd2 * dyv ** 2 - glo
    amax = np.exp(-np.maximum(D.min(axis=1), 0.0))
    keep = amax >= f32(1.0 / 255.0) - f32(1e-6)
    idx = cand[keep]
    if len(idx) == 0:
        return (np.zeros((6, 0), f32), np.zeros((0, 3), f32),
                bg.astype(f32).copy(), 0.0)
    # tile-centered quadratic coefficients
    x0f, y0f = f32(c0 + TC / 2.0), f32(r0 + TR / 2.0)
    ut, vt = u[idx] - x0f, v[idx] - y0f
    rr, gg, dd, lo = r[idx], g2[idx], d2[idx], logop[idx]
    st = ut + rr * vt
    coef = np.stack([gg,
                     2 * gg * rr,
                     gg * rr * rr + dd,
                     -2 * gg * st,
                     -2 * gg * rr * st - 2 * dd * vt,
                     gg * st * st + dd * vt * vt - lo], 0).astype(f32)
    col = cp["col"][idx]
    n = len(idx)
    dc = np.zeros((n, 3), f32)
    dc[:-1] = col[1:] - col[:-1]
    dc[-1] = bg - col[-1]
    return coef, dc, col[0].copy(), float(amax[keep].max())


# ------------------------------------------------------------- bass program
def _build_nc(struct):
    """struct: dict with
      slots: list of slots; each slot = list of padded tile lengths
      flags: per-slot bool (apply 0.99 clamp)
      novl:  total number of (chunk, tile) overlap color matmuls
      overlaps: per slot: list of (chunk_local_idx, col_lo, col_hi,
                 tile_idx_in_slot, ov_idx, is_first, is_last)
    """
    slots = struct["slots"]
    flags = struct["flags"]
    novl = struct["novl"]
    SL = sum(sum(s) for s in slots)
    nc = bacc.Bacc(None, target_bir_lowering=False)

    # cc packs [mono | chi_slot0 | clo_slot0 | chi_slot1 | clo_slot1 | ...]
    cc_d = nc.dram_tensor("cc", [6, 128 + 2 * SL], F16, kind="ExternalInput")
    ident_d = nc.dram_tensor("ident", [128, 128], F16, kind="ExternalInput")
    dcw_d = nc.dram_tensor("dcw", [128, 3 * novl], F16, kind="ExternalInput")
    img_d = nc.dram_tensor("img", [128, 3 * NPC], F32, kind="ExternalOutput")

    with tile.TileContext(nc) as tc, ExitStack() as ctx:
        consts = ctx.enter_context(tc.tile_pool(name="consts", bufs=1))
        apool = ctx.enter_context(tc.tile_pool(name="apool", bufs=2))
        tpool = ctx.enter_context(tc.tile_pool(name="tpool", bufs=2))
        ttspool = ctx.enter_context(tc.tile_pool(name="ttspool", bufs=3))
        outp = ctx.enter_context(tc.tile_pool(name="outp", bufs=2))
        dmmp = ctx.enter_context(tc.tile_pool(name="dmmp", bufs=2,
                                              space="PSUM"))
        tpp = ctx.enter_context(tc.tile_pool(name="tpp", bufs=3,
                                             space="PSUM"))
        imgp = ctx.enter_context(tc.tile_pool(name="imgp", bufs=1,
                                              space="PSUM"))

        cc = consts.tile([6, 128 + 2 * SL], F16)
        ident = consts.tile([128, 128], F16)
        dcw = consts.tile([128, 3 * novl], F16)
        mono = cc[:, 0:128]
        ccoffs = []      # per slot: start of its [chi | clo] block in cc
        off = 128
        for s in slots:
            ccoffs.append(off)
            off += 2 * sum(s)
        # staged input DMAs on the idle SP queue: slot 0's operands first,
        # then two bulk pieces; ident/dcw ride the gpsimd queue
        cuts = [0, ccoffs[0] + 2 * sum(slots[0]),
                ccoffs[min(4, len(slots)) - 1] + 2 * sum(
                    slots[min(4, len(slots)) - 1]), 128 + 2 * SL]
        for a, b in zip(cuts[:-1], cuts[1:]):
            if b > a:
                nc.sync.dma_start(cc[:, a:b], cc_d[:, a:b])
        nc.gpsimd.dma_start(ident[:], ident_d[:])
        nc.gpsimd.dma_start(dcw[:], dcw_d[:])
        zeros = consts.tile([128, SLOT_CAP], F16)
        nc.gpsimd.memset(zeros[:], 0.0)

        img_ps = imgp.tile([128, 3 * NPC], F32, name="img_ps")

        # prime the T buffers so transposes of partial chunks only ever see
        # finite values (psum garbage can be NaN; 0 * NaN = NaN in colors)
        for _ in range(2):
            tb0 = tpool.tile([128, SLOT_CAP], F16, tag="tbuf")
            nc.vector.memset(tb0[:], 0.0)

        copy_rot = [0]
        tbufs = {}

        def emit_dmm(si):
            Ls = sum(slots[si])
            so = ccoffs[si]
            dps = dmmp.tile([128, SLOT_CAP], F32, tag="dps")
            nc.tensor.matmul(dps[:, :Ls], mono, cc[:, so:so + Ls],
                             start=True, stop=False)
            nc.tensor.matmul(dps[:, :Ls], mono, cc[:, so + Ls:so + 2 * Ls],
                             start=False, stop=True)
            tbufs[si] = dict(dps=dps)

        def emit_transposes(si):
            st = tbufs[si]
            Ls = sum(slots[si])
            nch = -(-Ls // 128)
            tp = tpp.tile([128, 512], F16, tag="tp")
            for k in range(nch):
                nc.tensor.transpose(tp[:, k * 128:k * 128 + 128],
                                    st["tbuf"][:, k * 128:k * 128 + 128],
                                    ident[:])
            st["tp"] = tp
            st["nch"] = nch

        def emit_copy(si):
            st = tbufs[si]
            width = st["nch"] * 128
            tts = ttspool.tile([128, 512], F16, tag="tts")
            if copy_rot[0] % 4 == 3:
                nc.vector.tensor_copy(tts[:, :width], st["tp"][:, :width])
            else:
                nc.scalar.activation(tts[:, :width], st["tp"][:, :width],
                                     AF.Copy)
            copy_rot[0] += 1
            st["tts"] = tts

        def emit_exp_mask(si):
            st = tbufs[si]
            Ls = sum(slots[si])
            alpha = apool.tile([128, SLOT_CAP], F16, tag="alpha")
            nc.scalar.activation(alpha[:, :Ls], st["dps"][:, :Ls], AF.Exp,
                                 scale=-1.0)
            mcull = apool.tile([128, SLOT_CAP], F16, tag="mcull")
            nc.gpsimd.tensor_scalar(mcull[:, :Ls], alpha[:, :Ls],
                                    1.0 / 255.0, None, ALU.is_lt)
            na = apool.tile([128, SLOT_CAP], F16, tag="na")
            if flags[si]:
                nc.vector.tensor_scalar(na[:, :Ls], alpha[:, :Ls], -1.0,
                                        1.0, ALU.mult, ALU.add)
                nc.vector.tensor_scalar(na[:, :Ls], na[:, :Ls], 0.01, None,
                                        ALU.max)
            else:
                nc.vector.tensor_scalar(na[:, :Ls], alpha[:, :Ls], -1.0,
                                        1.0, ALU.mult, ALU.add)
            om = apool.tile([128, SLOT_CAP], F16, tag="om")
            nc.vector.tensor_tensor(om[:, :Ls], na[:, :Ls], mcull[:, :Ls],
                                    ALU.max)
            st["om"] = om

        def emit_scans(si):
            st = tbufs[si]
            tbuf = tpool.tile([128, SLOT_CAP], F16, tag="tbuf")
            toff = 0
            for Lp in slots[si]:
                nc.vector.tensor_tensor_scan(
                    tbuf[:, toff:toff + Lp], st["om"][:, toff:toff + Lp],
                    zeros[:, :Lp], 1.0, ALU.mult, ALU.add)
                toff += Lp
            st["tbuf"] = tbuf

        def emit_colors(si):
            st = tbufs[si]
            tts = st["tts"]
            for (ck, lo, hi, tj, ov, first, last) in struct["overlaps"][si]:
                gidx = struct["gidx"][si][tj]
                nc.tensor.matmul(
                    img_ps[:, 3 * gidx:3 * gidx + 3],
                    tts[:, ck * 128:ck * 128 + 128],
                    dcw[:, 3 * ov:3 * ov + 3],
                    start=first, stop=last)
            del st["dps"]

        def emit_drain(si_lo, si_hi):
            glo = struct["gidx"][si_lo][0]
            ghi = struct["gidx"][si_hi][-1] + 1
            ob = outp.tile([128, 3 * NPC], F32, tag="ob")
            nc.vector.tensor_copy(ob[:, 3 * glo:3 * ghi],
                                  img_ps[:, 3 * glo:3 * ghi])
            nc.sync.dma_start(img_d[:, 3 * glo:3 * ghi],
                              ob[:, 3 * glo:3 * ghi])

        n = len(slots)
        for si in range(n):
            emit_dmm(si)
            if si > 0:
                emit_transposes(si - 1)
                emit_copy(si - 1)
            emit_exp_mask(si)
            if si > 0:
                emit_colors(si - 1)
                if si % 2 == 0:
                    emit_drain(si - 2, si - 1)
            emit_scans(si)
        emit_transposes(n - 1)
        emit_copy(n - 1)
        emit_colors(n - 1)
        emit_drain(2 * ((n - 1) // 2), n - 1)

    saved = bacc.get_activation_tables
    bacc.get_activation_tables = _only_full_act_set
    try:
        nc.compile()
    finally:
        bacc.get_activation_tables = saved
    return nc


# ------------------------------------------------------------------ driver
def kernel(context_pose, target_poses, target_intrinsics, means1, means2,
           cov1, cov2, sh1, sh2, op1, op2, background_color,
           image_h, image_w):
    f32 = np.float32
    b, v = np.asarray(target_poses).shape[:2]
    assert b == 1 and v == NCAM and int(image_h) == H and int(image_w) == W

    context_pose = np.asarray(context_pose, f32)
    target_poses = np.asarray(target_poses, f32)
    target_intrinsics = np.asarray(target_intrinsics, f32)
    bg = np.asarray(background_color, f32)

    try:
        inv_base = np.linalg.inv(
            context_pose[0].astype(np.float64)).astype(f32)
    except np.linalg.LinAlgError:
        inv_base = np.linalg.pinv(
            context_pose[0].astype(np.float64)).astype(f32)
    d_sh = np.asarray(sh1).shape[-1]
    means = np.stack([np.asarray(means1, f32), np.asarray(means2, f32)],
                     1).reshape(-1, 3)
    covs = np.stack([np.asarray(cov1, f32), np.asarray(cov2, f32)],
                    1).reshape(-1, 3, 3)
    shs = np.stack([np.asarray(sh1, f32), np.asarray(sh2, f32)],
                   1).reshape(-1, 3, d_sh)
    ops = np.stack([np.asarray(op1, f32), np.asarray(op2, f32)],
                   1).reshape(-1)

    row_scale = np.array([1.0 / W, 1.0 / H, 1.0], f32)[:, None]
    cams = []
    for cam in range(NCAM):
        extr = inv_base @ target_poses[0, cam]
        Kn = target_intrinsics[0, cam] * row_scale
        K = np.array([[Kn[0, 0] * W, 0, Kn[0, 2] * W],
                      [0, Kn[1, 1] * H, Kn[1, 2] * H],
                      [0, 0, 1]], f32)
        cams.append(_prep_camera(extr, K, means, covs, shs, ops))
    assert all(c["psd"] for c in cams), "non-PSD conics unsupported in v4"

    # per-tile data
    tiles = []
    for cam in range(NCAM):
        for ty in range(NTY):
            for tx in range(NTX):
                coef, dc, c1, amax = _tile_data(cams[cam], ty, tx, bg)
                tiles.append(dict(cam=cam, ty=ty, tx=tx, coef=coef, dc=dc,
                                  c1=c1, amax=amax, L=coef.shape[1]))

    # snake assignment of size-sorted tiles to cores
    order = sorted(range(NTILE), key=lambda t: -tiles[t]["L"])
    percore = [[] for _ in range(8)]
    for k, t in enumerate(order):
        core = k % 8 if (k // 8) % 2 == 0 else 7 - (k % 8)
        percore[core].append(t)
    for core in range(8):
        percore[core].sort(key=lambda t: -tiles[t]["L"])

    # per-rank padded lengths (identical across cores)
    lpad = [max(1, max(tiles[percore[c][r]]["L"] for c in range(8)))
            for r in range(NPC)]

    # first-fit-decreasing into slots of <= SLOT_CAP columns
    slots_ranks, slots_len = [], []
    for r in range(NPC):
        placed = False
        for si in range(len(slots_ranks)):
            if slots_len[si] + lpad[r] <= SLOT_CAP:
                slots_ranks[si].append(r)
                slots_len[si] += lpad[r]
                placed = True
                break
        if not placed:
            slots_ranks.append([r])
            slots_len.append(lpad[r])
    # emission order: smallest slot first (fast pipeline prime), then the
    # rest descending, ending with the second-smallest (short tail)
    sizes = [sum(lpad[r] for r in ranks) for ranks in slots_ranks]
    asc = sorted(range(len(slots_ranks)), key=lambda i: sizes[i])
    if len(asc) > 2:
        emit_ord = ([asc[0]]
                    + sorted(asc[2:], key=lambda i: -sizes[i]) + [asc[1]])
    else:
        emit_ord = asc
    slots_ranks = [slots_ranks[i] for i in emit_ord]
    slots = [[lpad[r] for r in ranks] for ranks in slots_ranks]

    # clamp flags per slot (any core instance with alpha near/above 0.99)
    flags = []
    for ranks in slots_ranks:
        mx = max(tiles[percore[c][r]]["amax"]
                 for r in ranks for c in range(8))
        flags.append(bool(mx > 0.9895))

    # chunk overlap structure + global tile-slot indices
    overlaps, gidx = [], []
    g = 0
    ov = 0
    for si, s in enumerate(slots):
        gidx.append(list(range(g, g + len(s))))
        g += len(s)
        ovs = []
        toff = 0
        for tj, Lp in enumerate(s):
            lo, hi = toff, toff + Lp
            ck0, ck1 = lo // 128, (hi - 1) // 128
            for ck in range(ck0, ck1 + 1):
                a = max(lo, ck * 128)
                bnd = min(hi, ck * 128 + 128)
                ovs.append((ck, a, bnd, tj, ov, ck == ck0, ck == ck1))
                ov += 1
            toff += Lp
        overlaps.append(ovs)
    novl = ov
    struct = dict(slots=slots, flags=tuple(flags), novl=novl,
                  overlaps=overlaps, gidx=gidx)

    key = (tuple(tuple(s) for s in slots), tuple(flags), novl)
    if key not in _NC_CACHE:
        _NC_CACHE[key] = _build_nc(struct)
    nc = _NC_CACHE[key]

    # constants
    f16 = np.float16
    cvec = np.arange(TC, dtype=f32) - (TC / 2.0 - 0.5)
    rvec = np.arange(TR, dtype=f32) - (TR / 2.0 - 0.5)
    yyt, xxt = np.meshgrid(rvec, cvec, indexing="ij")
    xt, yt = xxt.reshape(-1), yyt.reshape(-1)      # [128] tile-local coords
    mono = np.stack([xt * xt, xt * yt, yt * yt, xt, yt,
                     np.ones(128, f32)], 0)
    mono16 = mono.astype(f16)
    assert np.all(mono16.astype(f32) == mono)
    ident = np.eye(128, dtype=f16)

    SL = sum(sum(s) for s in slots)
    in_maps = []
    for core in range(8):
        chi = np.zeros((6, SL), f32)
        chi[5, :] = PAD_F
        dcw = np.zeros((128, 3 * novl), f16)
        off = 0
        for si, s in enumerate(slots):
            toff = 0
            for tj, Lp in enumerate(s):
                t = tiles[percore[core][slots_ranks[si][tj]]]
                L = t["L"]
                chi[:, off + toff:off + toff + L] = t["coef"]
                toff += Lp
            for (ck, lo, hi, tj, ovi, first, last) in overlaps[si]:
                t = tiles[percore[core][slots_ranks[si][tj]]]
                L = t["L"]
                tstart = sum(s[:tj])
                r0 = lo - ck * 128
                for j in range(lo, hi):
                    gj = j - tstart
                    if gj < L:
                        dcw[r0 + (j - lo), 3 * ovi:3 * ovi + 3] = t["dc"][gj]
            off += sum(s)
        chi16 = chi.astype(f16)
        clo16 = (chi - chi16.astype(f32)).astype(f16)
        # pack [mono | chi_s0 | clo_s0 | chi_s1 | clo_s1 | ...]
        cc = np.zeros((6, 128 + 2 * SL), f16)
        cc[:, 0:128] = mono16
        off = 0
        ccoff = 128
        for s in slots:
            Ls = sum(s)
            cc[:, ccoff:ccoff + Ls] = chi16[:, off:off + Ls]
            cc[:, ccoff + Ls:ccoff + 2 * Ls] = clo16[:, off:off + Ls]
            off += Ls
            ccoff += 2 * Ls
        in_maps.append({"cc": cc, "ident": ident, "dcw": dcw})

    trace = os.environ.get("SPLAT_TRACE", "0") == "1"
    res = run_bass_kernel_spmd(nc, in_maps, core_ids=list(range(8)),
                               trace=trace,
                               trace_cores=list(range(8)) if trace else None)
    global _LAST_EXEC_NS, _LAST_RESULTS
    _LAST_EXEC_NS = res.exec_time_ns
    _LAST_RESULTS = res

    out = np.zeros((1, NCAM, 3, H, W), f32)
    for core in range(8):
        img = res.results[core]["img"]          # [128, 3*NPC]
        for si, ranks in enumerate(slots_ranks):
            for tj, r in enumerate(ranks):
                t = tiles[percore[core][r]]
                gg = gidx[si][tj]
                piece = img[:, 3 * gg:3 * gg + 3].T.reshape(3, TR, TC)
                cam, ty, tx = t["cam"], t["ty"], t["tx"]
                out[0, cam, :, ty * TR:(ty + 1) * TR,
                    tx * TC:(tx + 1) * TC] = piece + t["c1"][:, None, None]
    return out
